# revision 1
# baseline (speedup 1.0000x reference)
"""2-layer GAT (gnn_message_passing) on 8 TRN2 NeuronCores.

Strategy (graph/data parallel, per sharding hint):
  - Nodes are partitioned across 8 ranks (6250 dst nodes each). Each rank owns
    the segment-softmax + aggregation for its destination nodes.
  - Per layer, every rank computes the projected features (h = x @ W,
    attention source/dest logits al/ar fused into the same matmul via an
    augmented RHS) for ITS OWN nodes, writes them as rows of a gather table
    (768B rows for layer 1: 256 bf16 h + 8 f32 al; 256B rows for layer 2),
    then an AllGather replicates the full table to every rank.
  - Edge stage: destinations are degree-sorted and packed into tiles of 128
    (dst on partitions); each dst gets a padded run of incoming-edge "slots"
    along the free dimension. Source rows are fetched with dma_gather
    (SWDGE indexed gather). Since gather indices are int16, the table is
    addressed through two base windows (rows [0,32768) and
    [TROWS-32768, TROWS)), and each dst's slots are split into a "lo" and
    "hi" range accordingly.
  - Segment softmax is all free-dim math: e = leakyrelu(al_src + ar_dst) on
    the slot grid, p = exp(e) (no max-subtract needed at these magnitudes;
    mathematically identical), denom = free-dim reduce, normalization applied
    AFTER aggregation (divide the aggregated sums by denom).
  - Aggregation: msg = p (broadcast over channels) * h_src, then a pairwise
    tree of wide tensor adds along the slot dim.
  - Padding slots read a sentinel table row (h = 0, al = -1e30 -> p = 0).

The full output is assembled on the host from the 8 per-rank outputs
(undoing the degree-sort permutation).
"""

import math
import os
import sys
from contextlib import ExitStack
from dataclasses import dataclass, field

import numpy as np

for _p in ("/opt/trn_rl_repo",):
    if _p not in sys.path:
        sys.path.insert(0, _p)

import concourse.bass as bass
import concourse.bacc as bacc
import concourse.mybir as mybir
import concourse.tile as tile
from concourse import bass_utils

F32 = mybir.dt.float32
BF16 = mybir.dt.bfloat16
I16 = mybir.dt.int16
AL_SENT = -1.0e30
Alu = mybir.AluOpType
Act = mybir.ActivationFunctionType


@dataclass
class Cfg:
    N: int = 50000
    E: int = 500000          # edges before self-loops
    F_IN: int = 128
    HID: int = 32
    HEADS: int = 8
    OUT: int = 64
    NEG: float = 0.2
    R: int = 8
    SLOT_CAP: int = 32       # max slots per gather group (SBUF budget)
    hi_base: int = -1        # -1: auto (TROWS - 32768, clamped to >= 0)

    @property
    def HC1(self):
        return self.HEADS * self.HID     # 256

    @property
    def NPR(self):
        return self.N // self.R

    @property
    def CHUNK(self):
        return self.NPR + 1              # + sentinel row

    @property
    def TROWS(self):
        return self.R * self.CHUNK

    @property
    def T(self):
        return (self.NPR + 127) // 128   # dst tiles per rank

    @property
    def ROW1(self):
        return self.HC1 + 2 * self.HEADS + (128 - 2 * self.HEADS) % 128 \
            if False else 384            # bf16 elems: 256 h + 16 (8xf32 al) + pad

    @property
    def ROW2(self):
        return 128                       # bf16 elems: 64 h2 + 2 (1xf32 al2) + pad

    @property
    def HI_BASE(self):
        if self.hi_base >= 0:
            return self.hi_base
        return max(0, self.TROWS - 32768)

    @property
    def LO_LIM(self):
        # rows addressable from base 0
        return min(self.TROWS, 32768)


@dataclass
class Sched:
    perm: np.ndarray          # [R, NPR] perm[r][pos] = global node id
    sortpos: np.ndarray       # [N] position of node within its rank
    D_lo: np.ndarray          # [T]
    D_hi: np.ndarray          # [T]
    groups: list              # list of (t0, t1) tile ranges
    idx16: np.ndarray         # [R, 128, TOTCOL] int16
    call_cols: list           # per group: (lo_col0, lo_ncol, hi_col0, hi_ncol)
    sub_off: np.ndarray       # [T, 2] slot offset of (tile, region) inside its group buffer
    group_of: np.ndarray      # [T] group index of tile


def build_schedule(cfg: Cfg, src: np.ndarray, dst: np.ndarray) -> Sched:
    N, R, NPR, CHUNK, T = cfg.N, cfg.R, cfg.NPR, cfg.CHUNK, cfg.T
    deg = np.bincount(dst, minlength=N).astype(np.int64)

    rank_of = np.arange(N) // NPR
    sortpos = np.empty(N, np.int64)
    perm = np.empty((R, NPR), np.int64)
    for r in range(R):
        nodes = np.arange(r * NPR, (r + 1) * NPR)
        order = np.argsort(-deg[nodes], kind="stable")
        perm[r] = nodes[order]
        sortpos[perm[r]] = np.arange(NPR)
    # chunk row 0 of every rank is its sentinel row; real rows start at 1
    row_of = rank_of * CHUNK + 1 + sortpos        # [N] table row of each node

    src_row = row_of[src]
    # categories: 0 = forced lo, 1 = flexible, 2 = forced hi
    cat = np.where(src_row < cfg.HI_BASE, 0, np.where(src_row < cfg.LO_LIM, 1, 2))

    # global dst key: (rank, sorted position)
    dkey = rank_of[dst] * NPR + sortpos[dst]
    order = np.lexsort((cat, dkey))
    s_src_row = src_row[order]
    s_dkey = dkey[order]
    s_cat = cat[order]

    cnt = np.bincount(dkey, minlength=R * NPR)
    cnt_lo = np.bincount(dkey[cat == 0], minlength=R * NPR)
    cnt_hi = np.bincount(dkey[cat == 2], minlength=R * NPR)
    start = np.concatenate([[0], np.cumsum(cnt)])[:-1]

    # per-dst lo count: balance towards half, respecting forced counts
    half = (cnt + 1) // 2
    nlo = np.clip(half, cnt_lo, cnt - cnt_hi)
    nhi = cnt - nlo

    pos_in_dst = np.arange(len(order)) - start[s_dkey]
    is_lo = pos_in_dst < nlo[s_dkey]
    slot = np.where(is_lo, pos_in_dst, pos_in_dst - nlo[s_dkey])

    # tile schedule shared by all ranks
    tile_of_pos = np.arange(NPR) // 128
    D_lo = np.zeros(T, np.int64)
    D_hi = np.zeros(T, np.int64)
    nlo_g = nlo.reshape(R, NPR)
    nhi_g = nhi.reshape(R, NPR)
    for t in range(T):
        sl = slice(t * 128, min((t + 1) * 128, NPR))
        D_lo[t] = max(1, nlo_g[:, sl].max())
        D_hi[t] = max(1, nhi_g[:, sl].max())
    assert (D_lo + D_hi).max() <= cfg.SLOT_CAP, (
        f"tile needs {(D_lo + D_hi).max()} slots > SLOT_CAP {cfg.SLOT_CAP}")

    # greedy grouping of tiles, capped at SLOT_CAP slots
    groups = []
    group_of = np.zeros(T, np.int64)
    t0 = 0
    while t0 < T:
        t1 = t0 + 1
        tot = D_lo[t0] + D_hi[t0]
        while t1 < T and tot + D_lo[t1] + D_hi[t1] <= cfg.SLOT_CAP:
            tot += D_lo[t1] + D_hi[t1]
            t1 += 1
        group_of[t0:t1] = len(groups)
        groups.append((t0, t1))
        t0 = t1

    # slot offsets of each (tile, region) within its group buffer:
    # [lo slots of t0 | lo t1 | ... | hi t0 | hi t1 | ...]
    sub_off = np.zeros((T, 2), np.int64)
    call_cols = []
    col = 0
    pos_base_lo = np.zeros(T, np.int64)   # global gather-position base per tile region
    pos_base_hi = np.zeros(T, np.int64)
    for (t0, t1) in groups:
        S_lo = int(D_lo[t0:t1].sum())
        S_hi = int(D_hi[t0:t1].sum())
        off = 0
        lo_col0 = col
        for t in range(t0, t1):
            sub_off[t, 0] = off
            pos_base_lo[t] = col * 16 + off * 128
            off += D_lo[t]
        col += S_lo * 8  # 128/16 columns per slot-column
        hi_col0 = col
        off2 = 0
        for t in range(t0, t1):
            sub_off[t, 1] = S_lo + off2
            pos_base_hi[t] = col * 16 + off2 * 128
            off2 += D_hi[t]
        col += S_hi * 8
        call_cols.append((lo_col0, S_lo * 8, hi_col0, S_hi * 8))
    TOTCOL = col
    TOTPOS = TOTCOL * 16

    SENT_LO = 0                                    # rank 0 sentinel row
    SENT_HI = (R - 1) * CHUNK - cfg.HI_BASE        # last rank sentinel, local
    assert 0 <= SENT_HI < 32768

    # fill idx values per rank
    e_rank = s_dkey // NPR
    e_pos = s_dkey % NPR
    e_tile = e_pos // 128
    e_part = e_pos % 128
    idx16 = np.empty((R, 128, TOTCOL), np.int16)
    for r in range(R):
        vals = np.empty(TOTPOS, np.int32)
        # defaults: sentinels
        for (t0, t1), (lc0, lnc, hc0, hnc) in zip(groups, call_cols):
            vals[lc0 * 16:(lc0 + lnc) * 16] = SENT_LO
            vals[hc0 * 16:(hc0 + hnc) * 16] = SENT_HI
        m = (e_rank == r)
        mlo = m & is_lo
        mhi = m & ~is_lo
        p_lo = pos_base_lo[e_tile[mlo]] + slot[mlo] * 128 + e_part[mlo]
        vals[p_lo] = s_src_row[mlo]
        p_hi = pos_base_hi[e_tile[mhi]] + slot[mhi] * 128 + e_part[mhi]
        vals[p_hi] = s_src_row[mhi] - cfg.HI_BASE
        assert vals.min() >= 0 and vals.max() < 32768
        idx16[r] = np.tile(vals.reshape(-1, 16).T, (8, 1))

    return Sched(perm=perm, sortpos=sortpos, D_lo=D_lo, D_hi=D_hi,
                 groups=groups, idx16=idx16, call_cols=call_cols,
                 sub_off=sub_off, group_of=group_of)


def _bc(ap, shape):
    """broadcast an AP to shape (step-0 dims)"""
    return ap.broadcast_to(list(shape))


def build_program(cfg: Cfg, sch: Sched):
    """Build the single SPMD Bass program. Returns (nc, input_names)."""
    nc = bacc.Bacc("TRN2", target_bir_lowering=False, debug=False,
                   num_devices=cfg.R)
    T, NPR, CHUNK, TROWS = cfg.T, cfg.NPR, cfg.CHUNK, cfg.TROWS
    HC1, H, HID, OUT = cfg.HC1, cfg.HEADS, cfg.HID, cfg.OUT
    ROW1, ROW2 = cfg.ROW1, cfg.ROW2
    TOTCOL = sch.idx16.shape[2]
    NT = T * 128

    # ---- I/O ----
    xT = nc.dram_tensor("xT", [cfg.F_IN, NT], F32, kind="ExternalInput")
    idxs_d = nc.dram_tensor("idxs", [128, TOTCOL], I16, kind="ExternalInput")
    W1_d = nc.dram_tensor("W1", [cfg.F_IN, HC1], F32, kind="ExternalInput")
    W1T_d = nc.dram_tensor("W1T", [HC1, cfg.F_IN], F32, kind="ExternalInput")
    A1s_d = nc.dram_tensor("A1s", [HC1, H], F32, kind="ExternalInput")
    A1d_d = nc.dram_tensor("A1d", [HC1, H], F32, kind="ExternalInput")
    B1_d = nc.dram_tensor("B1rep", [128, HC1], F32, kind="ExternalInput")
    W2_d = nc.dram_tensor("W2", [HC1, OUT], F32, kind="ExternalInput")
    W2T_d = nc.dram_tensor("W2T", [OUT, HC1], F32, kind="ExternalInput")
    a2s_d = nc.dram_tensor("a2s", [OUT, 1], F32, kind="ExternalInput")
    a2d_d = nc.dram_tensor("a2d", [OUT, 1], F32, kind="ExternalInput")
    B2_d = nc.dram_tensor("B2rep", [128, OUT], F32, kind="ExternalInput")
    out_d = nc.dram_tensor("out", [NT, OUT], F32, kind="ExternalOutput")

    KC = HC1 // 128   # contraction chunks over HC1 (2)

    with tile.TileContext(nc) as tc, ExitStack() as ctx:
        dram = ctx.enter_context(tc.tile_pool(name="dram", bufs=1, space="DRAM"))
        const = ctx.enter_context(tc.tile_pool(name="const", bufs=1))
        psum = ctx.enter_context(tc.tile_pool(name="psum", bufs=2, space="PSUM"))

        # DRAM scratch
        chunk1 = dram.tile([CHUNK, ROW1], BF16)
        table1 = dram.tile([TROWS, ROW1], BF16, addr_space="Shared")
        chunk2 = dram.tile([CHUNK, ROW2], BF16)
        table2 = dram.tile([TROWS, ROW2], BF16, addr_space="Shared")
        h1d = dram.tile([NT, HC1], BF16)

        # ---- persistent constants ----
        idx_s = const.tile([128, TOTCOL], I16, tag="idx")
        nc.sync.dma_start(idx_s[:, :], idxs_d[:, :])
        RHS1 = const.tile([128, HC1 + 2 * H], F32, tag="rhs1")
        nc.sync.dma_start(RHS1[:, 0:HC1], W1_d[:, :])
        B1_s = const.tile([128, HC1], F32, tag="b1")
        nc.sync.dma_start(B1_s[:, :], B1_d[:, :])
        B2_s = const.tile([128, OUT], F32, tag="b2")
        nc.sync.dma_start(B2_s[:, :], B2_d[:, :])
        RHS2 = const.tile([128, KC, OUT + 2], BF16, tag="rhs2")
        nc.gpsimd.dma_start(RHS2[:, :, 0:OUT],
                            W2_d.ap().rearrange("(k p) c -> p k c", p=128))
        arL = const.tile([128, T, H], F32, tag="arL")
        nc.vector.memset(arL[:, :, :], 0.0)
        ar2L = const.tile([128, T, 1], F32, tag="ar2L")
        nc.vector.memset(ar2L[:, :, :], 0.0)

        # ================= phase 1: projection + table 1 ====================
        TS_T = (NPR + 1 + 127) // 128
        full_t = NPR // 128
        rem = NPR - full_t * 128
        with tc.tile_pool(name="ph1", bufs=1) as ph1:
            xT_s = ph1.tile([128, NT], F32, tag="xT")
            nc.sync.dma_start(xT_s[:, :], xT[:, :])
            W1T_s = ph1.tile([128, KC, 128], F32, tag="w1t")
            nc.sync.dma_start(W1T_s[:, :, :],
                              W1T_d.ap().rearrange("(k p) f -> p k f", p=128))
            A1s_s = ph1.tile([128, KC, H], F32, tag="a1s")
            nc.sync.dma_start(A1s_s[:, :, :],
                              A1s_d.ap().rearrange("(k p) h -> p k h", p=128))
            A1d_s = ph1.tile([128, KC, H], F32, tag="a1d")
            nc.sync.dma_start(A1d_s[:, :, :],
                              A1d_d.ap().rearrange("(k p) h -> p k h", p=128))
            W2T_s = ph1.tile([OUT, HC1], F32, tag="w2t")
            nc.sync.dma_start(W2T_s[:, :], W2T_d[:, :])
            a2s_s = ph1.tile([OUT, 1], F32, tag="a2s")
            nc.sync.dma_start(a2s_s[:, :], a2s_d[:, :])
            a2d_s = ph1.tile([OUT, 1], F32, tag="a2d")
            nc.sync.dma_start(a2d_s[:, :], a2d_d[:, :])

            # fold attention vectors into projection RHS
            for (dst_off, A_s) in ((HC1, A1s_s), (HC1 + H, A1d_s)):
                ps = psum.tile([128, H], F32, tag="wprep")
                for k in range(KC):
                    nc.tensor.matmul(ps[:, :], W1T_s[:, k, :], A_s[:, k, :],
                                     start=(k == 0), stop=(k == KC - 1))
                nc.vector.tensor_copy(RHS1[:, dst_off:dst_off + H], ps[:, :])
            for (dst_off, a_s) in ((OUT, a2s_s), (OUT + 1, a2d_s)):
                for k in range(KC):
                    ps = psum.tile([128, 1], F32, tag="wprep2")
                    nc.tensor.matmul(ps[:, :], W2T_s[:, k * 128:(k + 1) * 128],
                                     a_s[:, :], start=True, stop=True)
                    nc.vector.tensor_copy(RHS2[:, k, dst_off:dst_off + 1],
                                          ps[:, :])

            tstage = ph1.tile([128, TS_T, ROW1], BF16, tag="tstage1")
            nc.vector.memset(tstage[:, :, :], 0.0)
            for t in range(T):
                ps = psum.tile([128, HC1 + 2 * H], F32, tag="proj1")
                nc.tensor.matmul(ps[:, :], xT_s[:, t * 128:(t + 1) * 128],
                                 RHS1[:, :], start=True, stop=True)
                nc.scalar.copy(tstage[:, t, 0:HC1], ps[:, 0:HC1])
                al_view = tstage[:, t, HC1:HC1 + 2 * H].bitcast(F32)
                nc.vector.tensor_copy(al_view[:, :], ps[:, HC1:HC1 + H])
                nc.vector.tensor_copy(arL[:, t, :],
                                      ps[:, HC1 + H:HC1 + 2 * H])
            # sentinel row -> chunk row 0 (h = 0, al = -1e30)
            sent1 = ph1.tile([1, ROW1], BF16, tag="sent1")
            nc.vector.memset(sent1[:, :], 0.0)
            nc.vector.memset(sent1[:, HC1:HC1 + 2 * H].bitcast(F32), AL_SENT)
            nc.sync.dma_start(chunk1[0:1, :], sent1[:, :])
            nc.sync.dma_start(
                chunk1[1:1 + full_t * 128, :].rearrange("(t p) c -> p t c",
                                                        p=128),
                tstage[:, 0:full_t, :])
            if rem > 0:
                nc.sync.dma_start(chunk1[1 + full_t * 128:CHUNK, :],
                                  tstage[0:rem, full_t, :])
        nc.gpsimd.collective_compute(
            "AllGather", Alu.bypass,
            replica_groups=[list(range(cfg.R))],
            ins=[chunk1[:, :].opt()], outs=[table1[:, :].opt()])

        epool = ctx.enter_context(tc.tile_pool(name="edge", bufs=1))
        gpool = ctx.enter_context(tc.tile_pool(name="gpool", bufs=2))
        spool = ctx.enter_context(tc.tile_pool(name="spool", bufs=3))
        ppool = ctx.enter_context(tc.tile_pool(name="ppool", bufs=2))

        # ================= edge phase (shared for both layers) ==============
        def edge_layer(layer, table, ROW, CH, NH, arl_ap, bias_s, out_cb):
            """layer: 1 or 2. CH: channels per head (32 / 64). NH: heads.
            arl_ap(t) -> [128, NH] f32 AP; out_cb(t, unn, den) emits epilogue.
            """
            HCL = CH * NH
            lo_tab = table[0:cfg.LO_LIM, :]
            hi_tab = table[cfg.HI_BASE:TROWS, :]
            for gi, ((t0, t1), (lc0, lnc, hc0, hnc)) in enumerate(
                    zip(sch.groups, sch.call_cols)):
                S_lo = int(sch.D_lo[t0:t1].sum())
                S_hi = int(sch.D_hi[t0:t1].sum())
                S = S_lo + S_hi
                g = gpool.tile([128, cfg.SLOT_CAP, ROW], BF16, tag="gbuf")
                nc.gpsimd.dma_gather(
                    g[:, 0:S_lo, :], lo_tab, idx_s[:, lc0:lc0 + lnc],
                    num_idxs=S_lo * 128, num_idxs_reg=S_lo * 128,
                    elem_size=ROW, elem_step=ROW, single_packet=False)
                nc.gpsimd.dma_gather(
                    g[:, S_lo:S, :], hi_tab, idx_s[:, hc0:hc0 + hnc],
                    num_idxs=S_hi * 128, num_idxs_reg=S_hi * 128,
                    elem_size=ROW, elem_step=ROW, single_packet=False)

                for t in range(t0, t1):
                    parts, dens = [], []
                    for reg in (0, 1):
                        D = int((sch.D_lo, sch.D_hi)[reg][t])
                        so = int(sch.sub_off[t, reg])
                        gs = g[:, so:so + D, :]
                        # e = leakyrelu(al + ar)
                        e = spool.tile([128, cfg.SLOT_CAP, NH], F32, tag="e")
                        al = gs[:, :, HCL:HCL + 2 * NH].bitcast(F32)
                        nc.vector.tensor_add(
                            e[:, 0:D, :], al,
                            _bc(arl_ap(t).unsqueeze(1), (128, D, NH)))
                        nc.vector.scalar_tensor_tensor(
                            e[:, 0:D, :], e[:, 0:D, :], cfg.NEG, e[:, 0:D, :],
                            op0=Alu.mult, op1=Alu.max)
                        # p = exp(e)  (bf16 out)
                        p = spool.tile([128, cfg.SLOT_CAP, NH], BF16, tag="p")
                        nc.scalar.activation(p[:, 0:D, :], e[:, 0:D, :],
                                             Act.Exp)
                        # denom partial = sum over slots
                        den = spool.tile([128, NH], F32, tag="den")
                        nc.vector.tensor_reduce(
                            den[:, :], p[:, 0:D, :].transpose([0, 2, 1]),
                            axis=mybir.AxisListType.X, op=Alu.add)
                        dens.append(den)
                        # msg <- p broadcast over channels (doubling), then *= h
                        msg = ppool.tile([128, cfg.SLOT_CAP, NH, CH], BF16,
                                         tag="msg")
                        nc.vector.tensor_copy(msg[:, 0:D, :, 0:1],
                                              p[:, 0:D, :].unsqueeze(3))
                        k = 1
                        while k < CH:
                            kk = min(k, CH - k)
                            nc.vector.tensor_copy(msg[:, 0:D, :, k:k + kk],
                                                  msg[:, 0:D, :, 0:kk])
                            k += kk
                        msg = msg[:, :, :, :].rearrange("p d h c -> p d (h c)")
                        nc.vector.tensor_mul(msg[:, 0:D, :], msg[:, 0:D, :],
                                             gs[:, :, 0:HCL])
                        # tree-sum over slots -> part [128, HCL] f32
                        part = spool.tile([128, HCL], F32, tag="part")
                        cur = D
                        while cur > 2:
                            hh = cur // 2
                            nc.vector.tensor_add(
                                msg[:, 0:hh, :], msg[:, 0:hh, :],
                                msg[:, cur - hh:cur, :])
                            cur -= hh
                        if cur == 2:
                            nc.vector.tensor_add(part[:, :], msg[:, 0, :],
                                                 msg[:, 1, :])
                        else:
                            nc.vector.tensor_copy(part[:, :], msg[:, 0, :])
                        parts.append(part)
                    unn = spool.tile([128, HCL], F32, tag="unn")
                    nc.vector.tensor_add(unn[:, :], parts[0][:, :],
                                         parts[1][:, :])
                    den = spool.tile([128, NH], F32, tag="dent")
                    nc.vector.tensor_add(den[:, :], dens[0][:, :],
                                         dens[1][:, :])
                    nc.vector.tensor_scalar_add(den[:, :], den[:, :], 1e-16)
                    rec = spool.tile([128, NH], F32, tag="rec")
                    nc.vector.reciprocal(rec[:, :], den[:, :])
                    out_cb(t, unn, rec)

        # ---- L1 epilogue: normalize, +b1, ELU, store h1 ----
        def l1_out(t, unn, rec):
            y = spool.tile([128, HC1], F32, tag="y1")
            nc.vector.tensor_mul(
                y.rearrange("p (h c) -> p h c", h=H),
                unn.rearrange("p (h c) -> p h c", h=H),
                _bc(rec[:, :].unsqueeze(2), (128, H, HID)))
            nc.vector.tensor_add(y[:, :], y[:, :], B1_s[:, :])
            mn = spool.tile([128, HC1], F32, tag="mn1")
            nc.vector.tensor_scalar_min(mn[:, :], y[:, :], 0.0)
            nc.vector.tensor_scalar_max(y[:, :], y[:, :], 0.0)
            em = spool.tile([128, HC1], F32, tag="em1")
            nc.scalar.activation(em[:, :], mn[:, :], Act.Exp)
            h1t = spool.tile([128, HC1], BF16, tag="h1t")
            nc.vector.scalar_tensor_tensor(h1t[:, :], em[:, :], -1.0, y[:, :],
                                           op0=Alu.add, op1=Alu.add)
            nc.sync.dma_start(h1d[t * 128:(t + 1) * 128, :], h1t[:, :])

        edge_layer(1, table1, ROW1, HID, H, lambda t: arL[:, t, :], B1_s,
                   l1_out)

        # ---- L2 projection from h1 (DMA-transpose h1d) ----
        h1T = epool.tile([128, KC, NT], BF16, tag="h1T")
        for k in range(KC):
            nc.sync.dma_start_transpose(h1T[:, k, :],
                                        h1d[:, k * 128:(k + 1) * 128])
        tstage2 = epool.tile([128, TS_T, ROW2], BF16, tag="tstage2")
        nc.vector.memset(tstage2[:, :, :], 0.0)
        for t in range(T):
            ps = psum.tile([128, OUT + 2], F32, tag="proj2")
            for k in range(KC):
                nc.tensor.matmul(ps[:, :], h1T[:, k, t * 128:(t + 1) * 128],
                                 RHS2[:, k, :], start=(k == 0),
                                 stop=(k == KC - 1))
            nc.scalar.copy(tstage2[:, t, 0:OUT], ps[:, 0:OUT])
            al2_view = tstage2[:, t, OUT:OUT + 2].bitcast(F32)
            nc.vector.tensor_copy(al2_view[:, :], ps[:, OUT:OUT + 1])
            nc.vector.tensor_copy(ar2L[:, t, :], ps[:, OUT + 1:OUT + 2])
        sent2 = epool.tile([1, ROW2], BF16, tag="sent2")
        nc.vector.memset(sent2[:, :], 0.0)
        nc.vector.memset(sent2[:, OUT:OUT + 2].bitcast(F32), AL_SENT)
        nc.sync.dma_start(chunk2[0:1, :], sent2[:, :])
        nc.sync.dma_start(
            chunk2[1:1 + full_t * 128, :].rearrange("(t p) c -> p t c", p=128),
            tstage2[:, 0:full_t, :])
        if rem > 0:
            nc.sync.dma_start(chunk2[1 + full_t * 128:CHUNK, :],
                              tstage2[0:rem, full_t, :])
        nc.gpsimd.collective_compute(
            "AllGather", Alu.bypass,
            replica_groups=[list(range(cfg.R))],
            ins=[chunk2[:, :].opt()], outs=[table2[:, :].opt()])

        # ---- L2 epilogue: normalize, +b2, log_softmax, store out ----
        ostage = epool.tile([128, T, OUT], F32, tag="ostage")

        def l2_out(t, unn, rec):
            y = spool.tile([128, OUT], F32, tag="y2")
            nc.vector.tensor_scalar_mul(y[:, :], unn[:, :], rec[:, 0:1])
            nc.vector.tensor_add(y[:, :], y[:, :], B2_s[:, :])
            mx = spool.tile([128, 1], F32, tag="mx2")
            nc.vector.tensor_reduce(mx[:, :], y[:, :],
                                    axis=mybir.AxisListType.X, op=Alu.max)
            nc.vector.tensor_scalar_sub(y[:, :], y[:, :], mx[:, 0:1])
            ex = spool.tile([128, OUT], F32, tag="ex2")
            ssum = spool.tile([128, 1], F32, tag="ss2")
            nc.scalar.activation(ex[:, :], y[:, :], Act.Exp,
                                 accum_out=ssum[:, :])
            ls = spool.tile([128, 1], F32, tag="ls2")
            nc.scalar.activation(ls[:, :], ssum[:, :], Act.Ln)
            nc.vector.tensor_scalar_sub(ostage[:, t, :], y[:, :], ls[:, 0:1])

        edge_layer(2, table2, ROW2, OUT, 1, lambda t: ar2L[:, t, :], B2_s,
                   l2_out)
        nc.sync.dma_start(out_d.ap().rearrange("(t p) c -> p t c", p=128),
                          ostage[:, :, :])

    nc.compile()
    return nc


def _host_inputs(cfg: Cfg, sch: Sched, inputs: dict):
    """Build per-rank in_maps from the full problem inputs."""
    x = np.asarray(inputs["x"], np.float32)
    W1 = np.asarray(inputs["W1"], np.float32)
    a1_src = np.asarray(inputs["a1_src"], np.float32)
    a1_dst = np.asarray(inputs["a1_dst"], np.float32)
    b1 = np.asarray(inputs["b1"], np.float32)
    W2 = np.asarray(inputs["W2"], np.float32)
    a2_src = np.asarray(inputs["a2_src"], np.float32)
    a2_dst = np.asarray(inputs["a2_dst"], np.float32)
    b2 = np.asarray(inputs["b2"], np.float32)
    H, HID, HC1, OUT = cfg.HEADS, cfg.HID, cfg.HC1, cfg.OUT

    # block-diagonal per-head attention matrices: al = h @ A1s
    A1s = np.zeros((HC1, H), np.float32)
    A1d = np.zeros((HC1, H), np.float32)
    for h in range(H):
        A1s[h * HID:(h + 1) * HID, h] = a1_src[h]
        A1d[h * HID:(h + 1) * HID, h] = a1_dst[h]

    common = {
        "W1": np.ascontiguousarray(W1),
        "W1T": np.ascontiguousarray(W1.T),
        "A1s": A1s, "A1d": A1d,
        "B1rep": np.tile(b1[None, :], (128, 1)).astype(np.float32),
        "W2": np.ascontiguousarray(W2),
        "W2T": np.ascontiguousarray(W2.T),
        "a2s": np.ascontiguousarray(a2_src.reshape(OUT, 1)),
        "a2d": np.ascontiguousarray(a2_dst.reshape(OUT, 1)),
        "B2rep": np.tile(b2[None, :], (128, 1)).astype(np.float32),
    }
    in_maps = []
    for r in range(cfg.R):
        m = dict(common)
        xp = np.zeros((cfg.T * 128, x.shape[1]), np.float32)
        xp[:cfg.NPR] = x[sch.perm[r]]
        m["xT"] = np.ascontiguousarray(xp.T)
        m["idxs"] = np.ascontiguousarray(sch.idx16[r])
        in_maps.append(m)
    return in_maps


def run(cfg: Cfg, inputs: dict, trace: bool = False):
    edge_index = np.asarray(inputs["edge_index"])
    loops = np.arange(cfg.N, dtype=edge_index.dtype)
    src = np.concatenate([edge_index[0], loops]).astype(np.int64)
    dst = np.concatenate([edge_index[1], loops]).astype(np.int64)

    sch = build_schedule(cfg, src, dst)
    nc = build_program(cfg, sch)
    in_maps = _host_inputs(cfg, sch, inputs)
    res = bass_utils.run_bass_kernel_spmd(
        nc, in_maps, core_ids=list(range(cfg.R)), trace=trace)
    out = np.empty((cfg.N, cfg.OUT), np.float32)
    for r in range(cfg.R):
        o = res.results[r]["out"]
        out[sch.perm[r]] = o[:cfg.NPR]
    return out, res


def kernel(**inputs) -> np.ndarray:
    cfg = Cfg()
    out, _ = run(cfg, inputs)
    return out


if __name__ == "__main__":
    import reference
    inputs = {k: np.asarray(v) for k, v in reference.setup_inputs().items()}
    out = kernel(**inputs)
    exp = np.asarray(reference.reference(**reference.setup_inputs()))
    err = np.abs(out - exp).max() / (np.abs(exp).max() + 1e-12)
    print("rel err:", err)



# revision 9
# speedup vs baseline: 1.5273x; 1.5273x over previous
"""2-layer GAT (gnn_message_passing) on 8 TRN2 NeuronCores — v2.

Profiling of v1 showed the kernel is bound by GPSIMD (Q7) descriptor
generation for dma_gather at ~7.4 ns/descriptor; SDMA drain, HBM
bandwidth, PE and DVE all have headroom.  v2 minimizes gather
descriptors and removes the big AllGather:

  - PAIR-ROWS: table rows hold TWO consecutive nodes (1536 B for layer 1,
    512 B for layer 2), so a pair index fits int16 in ONE window
    (25088 < 32768).  The wrong half of each fetched pair is killed by a
    static {0,1} mask multiplied into the attention weight.  This removes
    the lo/hi window split of v1 and its E[max]+E[max] padding
    (870 -> ~507 slots in the tile schedule, -42% descriptors).
  - SELF-LOOPS are never gathered: each rank keeps its own projected rows
    (h + attention logits) resident in SBUF and adds the self
    contribution directly.
  - Layer-1 projection is REPLICATED: x is a replicated input, so every
    rank projects the full table locally; no AllGather for table 1 (the
    layer-2 table is still exchanged with a small ncfw AllGather).
  - Table rows store [h (c-major interleaved) | ones | al(f32)]: the ones
    channels make the softmax denominator fall out of the same tree-sum
    as the numerator, and the attention weight is broadcast-multiplied
    IN-PLACE into the gather buffer (no separate msg buffer, no channel
    broadcast copies).

Both layers share one slot/idx/mask schedule (same edges).
"""

import sys
from contextlib import ExitStack
from dataclasses import dataclass

import numpy as np
import ml_dtypes

for _p in ("/opt/trn_rl_repo",):
    if _p not in sys.path:
        sys.path.insert(0, _p)

import concourse.bass as bass
import concourse.bacc as bacc
import concourse.mybir as mybir
import concourse.tile as tile
from concourse import bass_utils

F32 = mybir.dt.float32
BF16 = mybir.dt.bfloat16
I16 = mybir.dt.int16
Alu = mybir.AluOpType
Act = mybir.ActivationFunctionType


@dataclass
class Cfg:
    N: int = 50000
    E: int = 500000
    F_IN: int = 128
    HID: int = 32
    HEADS: int = 8
    OUT: int = 64
    NEG: float = 0.2
    R: int = 8
    SLOT_CAP: int = 26

    @property
    def HC1(self):
        return self.HEADS * self.HID          # 256

    @property
    def NPR(self):
        return self.N // self.R               # 6250

    @property
    def T(self):
        return (self.NPR + 127) // 128        # 49 tiles per rank

    @property
    def CROWS(self):
        return self.T * 128                   # 6272 rows per rank chunk

    @property
    def TROWS(self):
        return self.R * self.CROWS            # 50176

    @property
    def ROW1(self):
        return 384                            # bf16 elems per node sub-row

    @property
    def ROW2(self):
        return 128


# ROW1 sub-row (bf16 elems): [0:256) h interleaved (elem 8c+h), [256:264)
# ones, [264:280) al per head as 8xf32, [280:384) pad.
# ROW2 sub-row: [0:64) h2, [64] one, [65] pad, [66:68) al2 f32, pad.


@dataclass
class Sched:
    perm: np.ndarray          # [R, NPR] global node id per (rank, pos)
    D: np.ndarray             # [T] slots per tile
    gso: np.ndarray           # [T] global slot offset of tile
    groups: list              # (t0, t1) with sum(D) <= SLOT_CAP
    sub_off: np.ndarray      # [T] slot offset within its group
    idx16: np.ndarray         # [R, 128, TOTCOL] int16 pair ids
    mask: np.ndarray          # [R, 128, TOTSLOT*2] bf16 {0,1}


def build_schedule(cfg: Cfg, src: np.ndarray, dst: np.ndarray) -> Sched:
    """src/dst: the raw edge list WITHOUT the appended self-loops (the
    kernel's own-row term supplies exactly one self-loop per node; natural
    (i,i) edges in the input stay in the list as regular slots)."""
    N, R, NPR, T = cfg.N, cfg.R, cfg.NPR, cfg.T
    es, ed = src, dst
    deg = np.bincount(ed, minlength=N).astype(np.int64)

    rank_of = np.arange(N) // NPR
    sortpos = np.empty(N, np.int64)
    perm = np.empty((R, NPR), np.int64)
    for r in range(R):
        nodes = np.arange(r * NPR, (r + 1) * NPR)
        order = np.argsort(-deg[nodes], kind="stable")
        perm[r] = nodes[order]
        sortpos[perm[r]] = np.arange(NPR)
    row_of = rank_of * cfg.CROWS + sortpos     # [N] global table row

    degs_sorted = deg[perm]                    # [R, NPR]
    D = np.zeros(T, np.int64)
    for t in range(T):
        sl = slice(t * 128, min((t + 1) * 128, NPR))
        D[t] = max(1, degs_sorted[:, sl].max())
    assert D.max() <= cfg.SLOT_CAP, f"{D.max()} > {cfg.SLOT_CAP}"
    gso = np.concatenate([[0], np.cumsum(D)])[:-1]
    TOTSLOT = int(D.sum())
    assert cfg.TROWS // 2 < 32768

    groups = []
    t0 = 0
    while t0 < T:
        t1, tot = t0 + 1, D[t0]
        while t1 < T and tot + D[t1] <= cfg.SLOT_CAP:
            tot += D[t1]
            t1 += 1
        groups.append((t0, t1))
        t0 = t1
    sub_off = np.array([gso[t] - gso[g0] for (g0, g1) in groups
                        for t in range(g0, g1)], np.int64)

    dkey = rank_of[ed] * NPR + sortpos[ed]
    order = np.argsort(dkey, kind="stable")
    s_row = row_of[es[order]]
    s_dkey = dkey[order]
    cnt = np.bincount(dkey, minlength=R * NPR)
    start = np.concatenate([[0], np.cumsum(cnt)])[:-1]
    slot = np.arange(len(order)) - start[s_dkey]

    e_rank = s_dkey // NPR
    e_pos = s_dkey % NPR
    e_tile = e_pos // 128
    e_part = e_pos % 128

    TOTCOL = TOTSLOT * 8
    idx16 = np.empty((R, 128, TOTCOL), np.int16)
    mask = np.zeros((R, 128, TOTSLOT * 2), np.float32)
    for r in range(R):
        vals = np.zeros(TOTSLOT * 128, np.int64)
        m = e_rank == r
        pos = gso[e_tile[m]] * 128 + slot[m] * 128 + e_part[m]
        vals[pos] = s_row[m] >> 1
        assert vals.max() < 32768
        idx16[r] = np.tile(vals.astype(np.int16).reshape(-1, 16).T, (8, 1))
        half = (s_row[m] & 1).astype(np.int64)
        mcol = (gso[e_tile[m]] + slot[m]) * 2 + half
        mask[r, e_part[m], mcol] = 1.0
    return Sched(perm=perm, D=D, gso=gso, groups=groups, sub_off=sub_off,
                 idx16=idx16,
                 mask=mask.astype(ml_dtypes.bfloat16))


def _bc(ap, shape):
    return ap.broadcast_to(list(shape))


def build_program(cfg: Cfg, sch: Sched):
    nc = bacc.Bacc("TRN2", target_bir_lowering=False, debug=False,
                   num_devices=cfg.R)
    T, R, TROWS, CROWS = cfg.T, cfg.R, cfg.TROWS, cfg.CROWS
    HC1, H, OUT = cfg.HC1, cfg.HEADS, cfg.OUT
    ROW1, ROW2 = cfg.ROW1, cfg.ROW2
    TOTCOL = sch.idx16.shape[2]
    TOTSLOT = sch.mask.shape[2] // 2
    NT = T * 128
    KC = HC1 // 128

    xT_d = nc.dram_tensor("xT", [cfg.F_IN, TROWS], BF16, kind="ExternalInput")
    xTo_d = nc.dram_tensor("xTo", [cfg.F_IN, CROWS], BF16,
                           kind="ExternalInput")
    idx_d = nc.dram_tensor("idxs", [128, TOTCOL], I16, kind="ExternalInput")
    msk_d = nc.dram_tensor("mask", [128, TOTSLOT * 2], BF16,
                           kind="ExternalInput")
    RHS1_d = nc.dram_tensor("RHS1", [128, HC1 + 2 * H], BF16,
                            kind="ExternalInput")
    RHS2_d = nc.dram_tensor("RHS2", [HC1, OUT + 2], BF16,
                            kind="ExternalInput")
    B1_d = nc.dram_tensor("B1rep", [128, HC1], F32, kind="ExternalInput")
    B2_d = nc.dram_tensor("B2rep", [128, OUT], F32, kind="ExternalInput")
    out_d = nc.dram_tensor("out", [NT, OUT], F32, kind="ExternalOutput")

    with tile.TileContext(nc) as tc, ExitStack() as ctx:
        dram = ctx.enter_context(tc.tile_pool(name="dram", bufs=1,
                                              space="DRAM"))
        const = ctx.enter_context(tc.tile_pool(name="const", bufs=1))
        psum = ctx.enter_context(tc.tile_pool(name="psum", bufs=2,
                                              space="PSUM"))

        table1 = dram.tile([TROWS, ROW1], BF16)
        h1d = dram.tile([NT, HC1], BF16)
        chunk2 = dram.tile([CROWS, ROW2], BF16)
        table2 = dram.tile([TROWS, ROW2], BF16, addr_space="Shared")

        idx_s = const.tile([128, TOTCOL], I16, tag="idx")
        nc.sync.dma_start(idx_s[:, :], idx_d[:, :])
        msk_s = const.tile([128, TOTSLOT * 2], BF16, tag="msk")
        nc.sync.dma_start(msk_s[:, :], msk_d[:, :])
        B1_s = const.tile([128, HC1], F32, tag="b1")
        nc.sync.dma_start(B1_s[:, :], B1_d[:, :])
        B2_s = const.tile([128, OUT], F32, tag="b2")
        nc.sync.dma_start(B2_s[:, :], B2_d[:, :])
        RHS1 = const.tile([128, HC1 + 2 * H], BF16, tag="rhs1")
        nc.sync.dma_start(RHS1[:, :], RHS1_d[:, :])
        RHS2 = const.tile([128, KC, OUT + 2], BF16, tag="rhs2")
        nc.sync.dma_start(RHS2[:, :, :],
                          RHS2_d.ap().rearrange("(k p) c -> p k c", p=128))

        # per-rank own-node state for self-loop contributions
        ownH1 = const.tile([128, T, 264], BF16, tag="ownh1")
        alL = const.tile([128, T, H], F32, tag="alL")
        arL = const.tile([128, T, H], F32, tag="arL")
        ownH2 = const.tile([128, T, 65], BF16, tag="ownh2")
        al2L = const.tile([128, T, 1], F32, tag="al2L")
        ar2L = const.tile([128, T, 1], F32, tag="ar2L")
        ostage = const.tile([128, T, OUT], F32, tag="ostage")

        # ============ phase 1: projections -> table 1 (replicated) ========
        with tc.tile_pool(name="ph1", bufs=2) as ph1:
            # own-chunk pass: fill ownH1 / alL / arL
            xo = ph1.tile([128, T, 128], BF16, tag="xo")
            nc.sync.dma_start(xo[:, :, :],
                              xTo_d.ap().rearrange("p (t n) -> p t n", n=128))
            nc.vector.memset(ownH1[:, :, 256:264], 1.0)
            for t in range(T):
                ps = psum.tile([128, HC1 + 2 * H], F32, tag="proj0")
                nc.tensor.matmul(ps[:, :], xo[:, t, :], RHS1[:, :],
                                 start=True, stop=True)
                nc.scalar.copy(ownH1[:, t, 0:HC1], ps[:, 0:HC1])
                nc.vector.tensor_copy(alL[:, t, :], ps[:, HC1:HC1 + H])
                nc.vector.tensor_copy(arL[:, t, :],
                                      ps[:, HC1 + H:HC1 + 2 * H])
            # full-table pass (all ranks compute everything)
            for q in range(R):
                xq = ph1.tile([128, T, 128], BF16, tag="xq")
                nc.sync.dma_start(
                    xq[:, :, :],
                    xT_d[:, q * CROWS:(q + 1) * CROWS]
                    .rearrange("p (t n) -> p t n", n=128))
                tst = ph1.tile([128, T, ROW1], BF16, tag="tst")
                nc.vector.memset(tst[:, :, 280:384], 0.0)
                nc.vector.memset(tst[:, :, 256:264], 1.0)
                for t in range(T):
                    ps = psum.tile([128, HC1 + 2 * H], F32, tag="proj1")
                    nc.tensor.matmul(ps[:, :], xq[:, t, :], RHS1[:, :],
                                     start=True, stop=True)
                    nc.scalar.copy(tst[:, t, 0:HC1], ps[:, 0:HC1])
                    al_v = tst[:, t, 264:280].bitcast(F32)
                    nc.vector.tensor_copy(al_v[:, :], ps[:, HC1:HC1 + H])
                nc.sync.dma_start(
                    table1[q * CROWS:(q + 1) * CROWS, :]
                    .rearrange("(t n) c -> n t c", n=128),
                    tst[:, :, :])

        pair1 = table1[:, :].rearrange("(a b) c -> a (b c)", b=2)
        pair2 = table2[:, :].rearrange("(a b) c -> a (b c)", b=2)

        spool = ctx.enter_context(tc.tile_pool(name="spool", bufs=3))

        # ================= edge phase (both layers) =================
        def edge_layer(gpool, pair_tab, ROW, NH, CHB, al_off, own, al_own,
                       ar_own, out_cb):
            """CHB: payload elems per sub-row (h+ones); al_off: f32-elem
            offset of al within a sub-row; own: [128, T, CHB] bf16."""
            for (t0, t1) in sch.groups:
                S = int(sch.D[t0:t1].sum())
                c0 = 8 * int(sch.gso[t0])
                g = gpool.tile([128, cfg.SLOT_CAP, ROW], BF16, tag="g")
                nc.gpsimd.dma_gather(
                    g[:, 0:S, :], pair_tab, idx_s[:, c0:c0 + 8 * S],
                    num_idxs=S * 128, num_idxs_reg=S * 128,
                    elem_size=ROW, elem_step=ROW, single_packet=False)
                for t in range(t0, t1):
                    D = int(sch.D[t])
                    DS = 2 * D
                    so = int(sch.gso[t] - sch.gso[t0])
                    gs = g[:, so:so + D, :]
                    al = gs.bitcast(F32).rearrange(
                        "p d (x c) -> p (d x) c", x=2)[:, :,
                                                       al_off:al_off + NH]
                    e = spool.tile([128, 2 * cfg.SLOT_CAP, NH], F32, tag="e")
                    nc.vector.tensor_add(
                        e[:, 0:DS, :], al,
                        _bc(ar_own(t).unsqueeze(1), (128, DS, NH)))
                    nc.vector.scalar_tensor_tensor(
                        e[:, 0:DS, :], e[:, 0:DS, :], cfg.NEG, e[:, 0:DS, :],
                        op0=Alu.mult, op1=Alu.max)
                    p = spool.tile([128, 2 * cfg.SLOT_CAP, NH], BF16, tag="p")
                    nc.scalar.activation(p[:, 0:DS, :], e[:, 0:DS, :],
                                         Act.Exp)
                    mk = msk_s[:, 2 * int(sch.gso[t]):
                               2 * (int(sch.gso[t]) + D)]
                    nc.vector.tensor_mul(
                        p[:, 0:DS, :], p[:, 0:DS, :],
                        _bc(mk.unsqueeze(2), (128, DS, NH)))
                    msg = gs.rearrange("p d (x c) -> p (d x) c", x=2)
                    msg4 = msg[:, :, 0:CHB].rearrange(
                        "p d (b h) -> p d b h", h=NH)
                    nc.vector.tensor_mul(
                        msg4, msg4,
                        _bc(p[:, 0:DS, :].unsqueeze(2),
                            (128, DS, CHB // NH, NH)))
                    # self contribution
                    es_ = spool.tile([128, NH], F32, tag="es")
                    nc.vector.tensor_add(es_[:, :], al_own(t), ar_own(t))
                    nc.vector.scalar_tensor_tensor(
                        es_[:, :], es_[:, :], cfg.NEG, es_[:, :],
                        op0=Alu.mult, op1=Alu.max)
                    ps_ = spool.tile([128, NH], F32, tag="psx")
                    nc.scalar.activation(ps_[:, :], es_[:, :], Act.Exp)
                    selfh = spool.tile([128, CHB], F32, tag="selfh")
                    nc.vector.tensor_copy(selfh[:, :], own[:, t, :])
                    part = spool.tile([128, CHB], F32, tag="part")
                    nc.vector.tensor_mul(
                        part[:, :].rearrange("p (b h) -> p b h", h=NH),
                        selfh[:, :].rearrange("p (b h) -> p b h", h=NH),
                        _bc(ps_[:, :].unsqueeze(1), (128, CHB // NH, NH)))
                    mv = msg[:, :, 0:CHB]
                    cur = DS
                    while cur > 1:
                        hh = cur // 2
                        nc.vector.tensor_add(mv[:, 0:hh, :], mv[:, 0:hh, :],
                                             mv[:, cur - hh:cur, :])
                        cur -= hh
                    tmp = spool.tile([128, CHB], F32, tag="tmp")
                    nc.vector.tensor_copy(tmp[:, :], mv[:, 0, :])
                    nc.vector.tensor_add(part[:, :], part[:, :], tmp[:, :])
                    out_cb(t, part)

        # ---- L1 epilogue ----
        def l1_out(t, part):
            part4 = part[:, :].rearrange("p (b h) -> p b h", h=H)
            den = part4[:, 32, :]
            nc.vector.tensor_scalar_add(den, den, 1e-16)
            rec = spool.tile([128, H], F32, tag="rec")
            nc.vector.reciprocal(rec[:, :], den)
            y = spool.tile([128, HC1], F32, tag="y1")
            nc.vector.tensor_mul(
                y[:, :].rearrange("p (b h) -> p b h", h=H),
                part4[:, 0:32, :], _bc(rec[:, :].unsqueeze(1), (128, 32, H)))
            nc.vector.tensor_add(y[:, :], y[:, :], B1_s[:, :])
            mn = spool.tile([128, HC1], F32, tag="mn1")
            nc.vector.tensor_scalar_min(mn[:, :], y[:, :], 0.0)
            nc.vector.tensor_scalar_max(y[:, :], y[:, :], 0.0)
            em = spool.tile([128, HC1], F32, tag="em1")
            nc.scalar.activation(em[:, :], mn[:, :], Act.Exp)
            h1t = spool.tile([128, HC1], BF16, tag="h1t")
            nc.vector.scalar_tensor_tensor(h1t[:, :], em[:, :], -1.0, y[:, :],
                                           op0=Alu.add, op1=Alu.add)
            nc.sync.dma_start(h1d[t * 128:(t + 1) * 128, :], h1t[:, :])

        with tc.tile_pool(name="gp1", bufs=2) as gp1:
            edge_layer(gp1, pair1, 2 * ROW1, H, 264, 132, ownH1,
                       lambda t: alL[:, t, :], lambda t: arL[:, t, :], l1_out)

        # ---- L2 projection (own rank only) + AllGather ----
        with tc.tile_pool(name="ph2", bufs=1) as ph2:
            h1T = ph2.tile([128, KC, NT], BF16, tag="h1T")
            for k in range(KC):
                nc.sync.dma_start_transpose(h1T[:, k, :],
                                            h1d[:, k * 128:(k + 1) * 128])
            tst2 = ph2.tile([128, T, ROW2], BF16, tag="tst2")
            nc.vector.memset(tst2[:, :, 64:128], 0.0)
            nc.vector.memset(tst2[:, :, 64:65], 1.0)
            nc.vector.memset(ownH2[:, :, 64:65], 1.0)
            for t in range(T):
                ps = psum.tile([128, OUT + 2], F32, tag="proj2")
                for k in range(KC):
                    nc.tensor.matmul(ps[:, :],
                                     h1T[:, k, t * 128:(t + 1) * 128],
                                     RHS2[:, k, :], start=(k == 0),
                                     stop=(k == KC - 1))
                nc.scalar.copy(tst2[:, t, 0:OUT], ps[:, 0:OUT])
                al2_v = tst2[:, t, 66:68].bitcast(F32)
                nc.vector.tensor_copy(al2_v[:, :], ps[:, OUT:OUT + 1])
                nc.vector.tensor_copy(al2L[:, t, :], ps[:, OUT:OUT + 1])
                nc.vector.tensor_copy(ar2L[:, t, :],
                                      ps[:, OUT + 1:OUT + 2])
                nc.scalar.copy(ownH2[:, t, 0:64], ps[:, 0:OUT])
            nc.sync.dma_start(
                chunk2[:, :].rearrange("(t n) c -> n t c", n=128),
                tst2[:, :, :])
        nc.gpsimd.collective_compute(
            "AllGather", Alu.bypass,
            replica_groups=[list(range(cfg.R))],
            ins=[chunk2[:, :].opt()], outs=[table2[:, :].opt()])

        # ---- L2 epilogue ----
        def l2_out(t, part):
            den = part[:, 64:65]
            nc.vector.tensor_scalar_add(den, den, 1e-16)
            rec = spool.tile([128, 1], F32, tag="rec2")
            nc.vector.reciprocal(rec[:, :], den)
            y = spool.tile([128, OUT], F32, tag="y2")
            nc.vector.tensor_scalar_mul(y[:, :], part[:, 0:OUT], rec[:, 0:1])
            nc.vector.tensor_add(y[:, :], y[:, :], B2_s[:, :])
            mx = spool.tile([128, 1], F32, tag="mx2")
            nc.vector.tensor_reduce(mx[:, :], y[:, :],
                                    axis=mybir.AxisListType.X, op=Alu.max)
            nc.vector.tensor_scalar_sub(y[:, :], y[:, :], mx[:, 0:1])
            ex = spool.tile([128, OUT], F32, tag="ex2")
            ssum = spool.tile([128, 1], F32, tag="ss2")
            nc.scalar.activation(ex[:, :], y[:, :], Act.Exp,
                                 accum_out=ssum[:, :])
            ls = spool.tile([128, 1], F32, tag="ls2")
            nc.scalar.activation(ls[:, :], ssum[:, :], Act.Ln)
            nc.vector.tensor_scalar_sub(ostage[:, t, :], y[:, :], ls[:, 0:1])

        with tc.tile_pool(name="gp2", bufs=2) as gp2:
            edge_layer(gp2, pair2, 2 * ROW2, 1, 65, 33, ownH2,
                       lambda t: al2L[:, t, :], lambda t: ar2L[:, t, :],
                       l2_out)
        nc.sync.dma_start(out_d.ap().rearrange("(t p) c -> p t c", p=128),
                          ostage[:, :, :])

    nc.compile()
    return nc


def _host_inputs(cfg: Cfg, sch: Sched, inputs: dict):
    x = np.asarray(inputs["x"], np.float32)
    W1 = np.asarray(inputs["W1"], np.float32)
    a1_src = np.asarray(inputs["a1_src"], np.float32)
    a1_dst = np.asarray(inputs["a1_dst"], np.float32)
    b1 = np.asarray(inputs["b1"], np.float32)
    W2 = np.asarray(inputs["W2"], np.float32)
    a2_src = np.asarray(inputs["a2_src"], np.float32)
    a2_dst = np.asarray(inputs["a2_dst"], np.float32)
    b2 = np.asarray(inputs["b2"], np.float32)
    H, HID, HC1, OUT = cfg.HEADS, cfg.HID, cfg.HC1, cfg.OUT

    il = np.empty(HC1, np.int64)               # il[8c+h] = h*32+c
    for c in range(HID):
        for h in range(H):
            il[8 * c + h] = h * HID + c

    Ws = np.zeros((cfg.F_IN, H), np.float32)
    Wd = np.zeros((cfg.F_IN, H), np.float32)
    for h in range(H):
        Ws[:, h] = W1[:, h * HID:(h + 1) * HID] @ a1_src[h]
        Wd[:, h] = W1[:, h * HID:(h + 1) * HID] @ a1_dst[h]
    RHS1 = np.concatenate([W1[:, il], Ws, Wd], axis=1)

    W2p = W2[il, :]
    vs = (W2 @ a2_src.reshape(OUT, 1))[il]
    vd = (W2 @ a2_dst.reshape(OUT, 1))[il]
    RHS2 = np.concatenate([W2p, vs, vd], axis=1)

    xT = np.zeros((cfg.F_IN, cfg.TROWS), np.float32)
    for r in range(cfg.R):
        xT[:, r * cfg.CROWS:r * cfg.CROWS + cfg.NPR] = x[sch.perm[r]].T
    xT16 = xT.astype(ml_dtypes.bfloat16)

    common = {
        "xT": xT16,
        "RHS1": np.ascontiguousarray(RHS1).astype(ml_dtypes.bfloat16),
        "RHS2": np.ascontiguousarray(RHS2).astype(ml_dtypes.bfloat16),
        "B1rep": np.tile(b1[il][None, :], (128, 1)).astype(np.float32),
        "B2rep": np.tile(b2[None, :], (128, 1)).astype(np.float32),
    }
    in_maps = []
    for r in range(cfg.R):
        m = dict(common)
        m["xTo"] = np.ascontiguousarray(
            xT16[:, r * cfg.CROWS:(r + 1) * cfg.CROWS])
        m["idxs"] = np.ascontiguousarray(sch.idx16[r])
        m["mask"] = np.ascontiguousarray(sch.mask[r])
        in_maps.append(m)
    return in_maps


def run(cfg: Cfg, inputs: dict, trace: bool = False):
    edge_index = np.asarray(inputs["edge_index"])
    src = edge_index[0].astype(np.int64)
    dst = edge_index[1].astype(np.int64)

    sch = build_schedule(cfg, src, dst)
    nc = build_program(cfg, sch)
    in_maps = _host_inputs(cfg, sch, inputs)
    res = bass_utils.run_bass_kernel_spmd(
        nc, in_maps, core_ids=list(range(cfg.R)), trace=trace)
    out = np.empty((cfg.N, cfg.OUT), np.float32)
    for r in range(cfg.R):
        o = np.asarray(res.results[r]["out"], np.float32)
        out[sch.perm[r]] = o[:cfg.NPR]
    return out, res


def kernel(**inputs) -> np.ndarray:
    cfg = Cfg()
    out, _ = run(cfg, inputs)
    return out


if __name__ == "__main__":
    import reference
    inputs = {k: np.asarray(v) for k, v in reference.setup_inputs().items()}
    out = kernel(**inputs)
    exp = np.asarray(reference.reference(**reference.setup_inputs()))
    err = np.abs(out - exp).max() / (np.abs(exp).max() + 1e-12)
    print("rel err:", err)


# revision 14
# speedup vs baseline: 1.5640x; 1.0240x over previous
"""2-layer GAT (gnn_message_passing) on 8 TRN2 NeuronCores — v2.

Profiling of v1 showed the kernel is bound by GPSIMD (Q7) descriptor
generation for dma_gather at ~7.4 ns/descriptor; SDMA drain, HBM
bandwidth, PE and DVE all have headroom.  v2 minimizes gather
descriptors and removes the big AllGather:

  - PAIR-ROWS: table rows hold TWO consecutive nodes (1536 B for layer 1,
    512 B for layer 2), so a pair index fits int16 in ONE window
    (25088 < 32768).  The wrong half of each fetched pair is killed by a
    static {0,1} mask multiplied into the attention weight.  This removes
    the lo/hi window split of v1 and its E[max]+E[max] padding
    (870 -> ~507 slots in the tile schedule, -42% descriptors).
  - SELF-LOOPS are never gathered: each rank keeps its own projected rows
    (h + attention logits) resident in SBUF and adds the self
    contribution directly.
  - Layer-1 projection is REPLICATED: x is a replicated input, so every
    rank projects the full table locally; no AllGather for table 1 (the
    layer-2 table is still exchanged with a small ncfw AllGather).
  - Table rows store [h (c-major interleaved) | ones | al(f32)]: the ones
    channels make the softmax denominator fall out of the same tree-sum
    as the numerator, and the attention weight is broadcast-multiplied
    IN-PLACE into the gather buffer (no separate msg buffer, no channel
    broadcast copies).

Both layers share one slot/idx/mask schedule (same edges).
"""

import sys
from contextlib import ExitStack
from dataclasses import dataclass

import numpy as np
import ml_dtypes

for _p in ("/opt/trn_rl_repo",):
    if _p not in sys.path:
        sys.path.insert(0, _p)

import concourse.bass as bass
import concourse.bacc as bacc
import concourse.mybir as mybir
import concourse.tile as tile
from concourse import bass_utils

F32 = mybir.dt.float32
BF16 = mybir.dt.bfloat16
I16 = mybir.dt.int16
Alu = mybir.AluOpType
Act = mybir.ActivationFunctionType


@dataclass
class Cfg:
    N: int = 50000
    E: int = 500000
    F_IN: int = 128
    HID: int = 32
    HEADS: int = 8
    OUT: int = 64
    NEG: float = 0.2
    R: int = 8
    SLOT_CAP: int = 28
    GMAX: int = 4

    @property
    def HC1(self):
        return self.HEADS * self.HID          # 256

    @property
    def NPR(self):
        return self.N // self.R               # 6250

    @property
    def T(self):
        return (self.NPR + 127) // 128        # 49 tiles per rank

    @property
    def CROWS(self):
        return self.T * 128                   # 6272 rows per rank chunk

    @property
    def TROWS(self):
        return self.R * self.CROWS            # 50176

    @property
    def ROW1(self):
        return 384                            # bf16 elems per node sub-row

    @property
    def ROW2(self):
        return 128


# ROW1 sub-row (bf16 elems): [0:256) h interleaved (elem 8c+h), [256:264)
# ones, [264:272) al per head bf16, [272:384) pad.
# ROW2 sub-row: [0:64) h2, [64] one, [65] pad, [66] al2 bf16, pad.


@dataclass
class Sched:
    perm: np.ndarray          # [R, NPR] global node id per (rank, pos)
    D: np.ndarray             # [T] slots per tile
    gso: np.ndarray           # [T] global slot offset of tile
    groups: list              # (t0, t1) with sum(D) <= SLOT_CAP
    sub_off: np.ndarray      # [T] slot offset within its group
    idx16: np.ndarray         # [R, 128, TOTCOL] int16 pair ids
    mask: np.ndarray          # [R, 128, TOTSLOT*2] bf16 {0,1}


def build_schedule(cfg: Cfg, src: np.ndarray, dst: np.ndarray) -> Sched:
    """src/dst: the raw edge list WITHOUT the appended self-loops (the
    kernel's own-row term supplies exactly one self-loop per node; natural
    (i,i) edges in the input stay in the list as regular slots)."""
    N, R, NPR, T = cfg.N, cfg.R, cfg.NPR, cfg.T
    es, ed = src, dst
    deg = np.bincount(ed, minlength=N).astype(np.int64)

    rank_of = np.arange(N) // NPR
    sortpos = np.empty(N, np.int64)
    perm = np.empty((R, NPR), np.int64)
    for r in range(R):
        nodes = np.arange(r * NPR, (r + 1) * NPR)
        order = np.argsort(-deg[nodes], kind="stable")
        perm[r] = nodes[order]
        sortpos[perm[r]] = np.arange(NPR)
    row_of = rank_of * cfg.CROWS + sortpos     # [N] global table row

    degs_sorted = deg[perm]                    # [R, NPR]
    Dt = np.zeros(T, np.int64)
    for t in range(T):
        sl = slice(t * 128, min((t + 1) * 128, NPR))
        Dt[t] = max(1, degs_sorted[:, sl].max())
    assert Dt.max() <= cfg.SLOT_CAP, f"{Dt.max()} > {cfg.SLOT_CAP}"
    assert cfg.TROWS // 2 < 32768

    # equal-D batches: G consecutive tiles padded to the first tile's D so
    # one vector instruction can span all G tiles with regular APs
    groups = []
    D = np.zeros(T, np.int64)
    t0 = 0
    while t0 < T:
        Dm, G = Dt[t0], 1
        while (t0 + G < T and G < cfg.GMAX
               and (G + 1) * Dm <= cfg.SLOT_CAP):
            G += 1
        D[t0:t0 + G] = Dm
        groups.append((t0, t0 + G))
        t0 += G
    gso = np.concatenate([[0], np.cumsum(D)])[:-1]
    TOTSLOT = int(D.sum())
    sub_off = np.array([gso[t] - gso[g0] for (g0, g1) in groups
                        for t in range(g0, g1)], np.int64)

    dkey = rank_of[ed] * NPR + sortpos[ed]
    order = np.argsort(dkey, kind="stable")
    s_row = row_of[es[order]]
    s_dkey = dkey[order]
    cnt = np.bincount(dkey, minlength=R * NPR)
    start = np.concatenate([[0], np.cumsum(cnt)])[:-1]
    slot = np.arange(len(order)) - start[s_dkey]

    e_rank = s_dkey // NPR
    e_pos = s_dkey % NPR
    e_tile = e_pos // 128
    e_part = e_pos % 128

    TOTCOL = TOTSLOT * 8
    idx16 = np.empty((R, 128, TOTCOL), np.int16)
    mask = np.zeros((R, 128, TOTSLOT * 2), np.float32)
    for r in range(R):
        vals = np.zeros(TOTSLOT * 128, np.int64)
        m = e_rank == r
        pos = gso[e_tile[m]] * 128 + slot[m] * 128 + e_part[m]
        vals[pos] = s_row[m] >> 1
        assert vals.max() < 32768
        idx16[r] = np.tile(vals.astype(np.int16).reshape(-1, 16).T, (8, 1))
        half = (s_row[m] & 1).astype(np.int64)
        mcol = (gso[e_tile[m]] + slot[m]) * 2 + half
        mask[r, e_part[m], mcol] = 1.0
    return Sched(perm=perm, D=D, gso=gso, groups=groups, sub_off=sub_off,
                 idx16=idx16,
                 mask=mask.astype(ml_dtypes.bfloat16))


def _bc(ap, shape):
    return ap.broadcast_to(list(shape))


def build_program(cfg: Cfg, sch: Sched):
    nc = bacc.Bacc("TRN2", target_bir_lowering=False, debug=False,
                   num_devices=cfg.R)
    T, R, TROWS, CROWS = cfg.T, cfg.R, cfg.TROWS, cfg.CROWS
    HC1, H, OUT = cfg.HC1, cfg.HEADS, cfg.OUT
    ROW1, ROW2 = cfg.ROW1, cfg.ROW2
    TOTCOL = sch.idx16.shape[2]
    TOTSLOT = sch.mask.shape[2] // 2
    NT = T * 128
    KC = HC1 // 128
    G_MAX = cfg.GMAX
    CAP2 = cfg.SLOT_CAP

    xT_d = nc.dram_tensor("xT", [cfg.F_IN, TROWS], BF16, kind="ExternalInput")
    xTo_d = nc.dram_tensor("xTo", [cfg.F_IN, CROWS], BF16,
                           kind="ExternalInput")
    idx_d = nc.dram_tensor("idxs", [128, TOTCOL], I16, kind="ExternalInput")
    msk_d = nc.dram_tensor("mask", [128, TOTSLOT * 2], BF16,
                           kind="ExternalInput")
    RHS1_d = nc.dram_tensor("RHS1", [128, HC1 + 2 * H], BF16,
                            kind="ExternalInput")
    RHS2_d = nc.dram_tensor("RHS2", [HC1, OUT + 2], BF16,
                            kind="ExternalInput")
    B1_d = nc.dram_tensor("B1rep", [128, HC1], F32, kind="ExternalInput")
    B2_d = nc.dram_tensor("B2rep", [128, OUT], F32, kind="ExternalInput")
    out_d = nc.dram_tensor("out", [NT, OUT], F32, kind="ExternalOutput")

    with tile.TileContext(nc) as tc, ExitStack() as ctx:
        dram = ctx.enter_context(tc.tile_pool(name="dram", bufs=1,
                                              space="DRAM"))
        const = ctx.enter_context(tc.tile_pool(name="const", bufs=1))
        psum = ctx.enter_context(tc.tile_pool(name="psum", bufs=2,
                                              space="PSUM"))

        table1 = dram.tile([TROWS, ROW1], BF16)
        h1d = dram.tile([NT, HC1], BF16)
        chunk2 = dram.tile([CROWS, ROW2], BF16)
        table2 = dram.tile([TROWS, ROW2], BF16, addr_space="Shared")

        idx_s = const.tile([128, TOTCOL], I16, tag="idx")
        nc.sync.dma_start(idx_s[:, :], idx_d[:, :])
        msk_s = const.tile([128, TOTSLOT * 2], BF16, tag="msk")
        nc.sync.dma_start(msk_s[:, :], msk_d[:, :])
        B1_s = const.tile([128, HC1], F32, tag="b1")
        nc.sync.dma_start(B1_s[:, :], B1_d[:, :])
        B2_s = const.tile([128, OUT], F32, tag="b2")
        nc.sync.dma_start(B2_s[:, :], B2_d[:, :])
        RHS1 = const.tile([128, HC1 + 2 * H], BF16, tag="rhs1")
        nc.sync.dma_start(RHS1[:, :], RHS1_d[:, :])
        RHS2 = const.tile([128, KC, OUT + 2], BF16, tag="rhs2")
        nc.sync.dma_start(RHS2[:, :, :],
                          RHS2_d.ap().rearrange("(k p) c -> p k c", p=128))

        # per-rank own-node state for self-loop contributions
        ownH1 = const.tile([128, T, 264], BF16, tag="ownh1")
        alL = const.tile([128, T, H], BF16, tag="alL")
        arL = const.tile([128, T, H], BF16, tag="arL")
        ownH2 = const.tile([128, T, 65], BF16, tag="ownh2")
        al2L = const.tile([128, T, 1], BF16, tag="al2L")
        ar2L = const.tile([128, T, 1], BF16, tag="ar2L")
        ostage = const.tile([128, T, OUT], F32, tag="ostage")

        # ============ phase 1: projections -> table 1 (replicated) ========
        with tc.tile_pool(name="ph1", bufs=2) as ph1:
            # own-chunk pass: fill ownH1 / alL / arL
            xo = ph1.tile([128, T, 128], BF16, tag="xo")
            nc.sync.dma_start(xo[:, :, :],
                              xTo_d.ap().rearrange("p (t n) -> p t n", n=128))
            nc.vector.memset(ownH1[:, :, 256:264], 1.0)
            for t in range(T):
                ps = psum.tile([128, HC1 + 2 * H], F32, tag="proj0")
                nc.tensor.matmul(ps[:, :], xo[:, t, :], RHS1[:, :],
                                 start=True, stop=True)
                nc.scalar.copy(ownH1[:, t, 0:HC1], ps[:, 0:HC1])
                nc.vector.tensor_copy(alL[:, t, :], ps[:, HC1:HC1 + H])
                nc.vector.tensor_copy(arL[:, t, :],
                                      ps[:, HC1 + H:HC1 + 2 * H])
            # full-table pass (all ranks compute everything)
            for q in range(R):
                xq = ph1.tile([128, T, 128], BF16, tag="xq")
                nc.sync.dma_start(
                    xq[:, :, :],
                    xT_d[:, q * CROWS:(q + 1) * CROWS]
                    .rearrange("p (t n) -> p t n", n=128))
                tst = ph1.tile([128, T, ROW1], BF16, tag="tst")
                nc.vector.memset(tst[:, :, 272:384], 0.0)
                nc.vector.memset(tst[:, :, 256:264], 1.0)
                for t in range(T):
                    ps = psum.tile([128, HC1 + 2 * H], F32, tag="proj1")
                    nc.tensor.matmul(ps[:, :], xq[:, t, :], RHS1[:, :],
                                     start=True, stop=True)
                    nc.scalar.copy(tst[:, t, 0:HC1], ps[:, 0:HC1])
                    nc.scalar.copy(tst[:, t, 264:272], ps[:, HC1:HC1 + H])
                nc.sync.dma_start(
                    table1[q * CROWS:(q + 1) * CROWS, :]
                    .rearrange("(t n) c -> n t c", n=128),
                    tst[:, :, :])

        pair1 = table1[:, :].rearrange("(a b) c -> a (b c)", b=2)
        pair2 = table2[:, :].rearrange("(a b) c -> a (b c)", b=2)

        spool = ctx.enter_context(tc.tile_pool(name="spool", bufs=2))

        # ================= edge phase (both layers) =================
        def edge_layer(gpool, CAP, pair_tab, ROW, NH, CHB, al_off, own,
                       alo, aro, out_cb):
            """Batched over equal-D tile groups.  ROW: pair elems; CHB:
            payload elems per sub-row (h+ones); al_off: bf16-elem offset of
            al in a sub-row; own: [128, T, CHB] bf16; alo/aro: [128, T, NH]
            bf16."""
            SUB = ROW // 2
            for (t0, t1) in sch.groups:
                G = t1 - t0
                Dm = int(sch.D[t0])
                S = G * Dm                    # gathered pairs
                S2, D2 = 2 * S, 2 * Dm
                b0 = int(sch.gso[t0])
                g = gpool.tile([128, CAP, ROW], BF16, tag="g")
                nc.gpsimd.dma_gather(
                    g[:, 0:S, :], pair_tab, idx_s[:, 8 * b0:8 * (b0 + S)],
                    num_idxs=S * 128, num_idxs_reg=S * 128,
                    elem_size=ROW, elem_step=ROW, single_packet=False)
                gs = g[:, 0:S, :]
                sub = gs.rearrange("p s (x c) -> p (s x) c", x=2)
                al = sub[:, :, al_off:al_off + NH]
                e = spool.tile([128, 2 * CAP, NH], F32, tag="e")
                nc.vector.tensor_add(
                    e[:, 0:S2, :].rearrange("p (g d) h -> p g d h", g=G),
                    al.rearrange("p (g d) h -> p g d h", g=G),
                    _bc(aro[:, t0:t1, :].unsqueeze(2), (128, G, D2, NH)))
                nc.vector.scalar_tensor_tensor(
                    e[:, 0:S2, :], e[:, 0:S2, :], cfg.NEG, e[:, 0:S2, :],
                    op0=Alu.mult, op1=Alu.max)
                p = spool.tile([128, 2 * CAP, NH], BF16, tag="p")
                nc.scalar.activation(p[:, 0:S2, :], e[:, 0:S2, :], Act.Exp)
                mk = msk_s[:, 2 * b0:2 * (b0 + S)]
                nc.vector.tensor_mul(
                    p[:, 0:S2, :], p[:, 0:S2, :],
                    _bc(mk.unsqueeze(2), (128, S2, NH)))
                msg4 = sub[:, :, 0:CHB].rearrange("p s (b h) -> p s b h",
                                                  h=NH)
                nc.vector.tensor_mul(
                    msg4, msg4,
                    _bc(p[:, 0:S2, :].unsqueeze(2),
                        (128, S2, CHB // NH, NH)))
                # self contribution
                es_ = spool.tile([128, G_MAX, NH], F32, tag="es")
                nc.vector.tensor_add(es_[:, 0:G, :], alo[:, t0:t1, :],
                                     aro[:, t0:t1, :])
                nc.vector.scalar_tensor_tensor(
                    es_[:, 0:G, :], es_[:, 0:G, :], cfg.NEG, es_[:, 0:G, :],
                    op0=Alu.mult, op1=Alu.max)
                ps_ = spool.tile([128, G_MAX, NH], F32, tag="psx")
                nc.scalar.activation(ps_[:, 0:G, :], es_[:, 0:G, :], Act.Exp)
                selfh = spool.tile([128, G_MAX, CHB], F32, tag="selfh")
                nc.vector.tensor_copy(selfh[:, 0:G, :], own[:, t0:t1, :])
                part = spool.tile([128, G_MAX, CHB], F32, tag="part")
                nc.vector.tensor_mul(
                    part[:, 0:G, :].rearrange("p g (b h) -> p g b h", h=NH),
                    selfh[:, 0:G, :].rearrange("p g (b h) -> p g b h", h=NH),
                    _bc(ps_[:, 0:G, :].unsqueeze(2),
                        (128, G, CHB // NH, NH)))
                # tree-sum over D2 slots within each tile, all tiles at once
                mv = sub[:, :, 0:CHB].rearrange("p (g d) c -> p g d c", g=G)
                cur = D2
                while cur > 1:
                    hh = cur // 2
                    nc.vector.tensor_add(mv[:, :, 0:hh, :], mv[:, :, 0:hh, :],
                                         mv[:, :, cur - hh:cur, :])
                    cur -= hh
                tmp = spool.tile([128, G_MAX, CHB], F32, tag="tmp")
                nc.vector.tensor_copy(tmp[:, 0:G, :], mv[:, :, 0, :])
                nc.vector.tensor_add(part[:, 0:G, :], part[:, 0:G, :],
                                     tmp[:, 0:G, :])
                out_cb(t0, G, part)

        # ---- L1 epilogue (batched) ----
        def l1_out(t0, G, part):
            part4 = part[:, 0:G, :].rearrange("p g (b h) -> p g b h", h=H)
            den = part4[:, :, 32, :]
            rec = spool.tile([128, G_MAX, H], F32, tag="rec")
            nc.vector.reciprocal(rec[:, 0:G, :], den)
            y = spool.tile([128, G_MAX, HC1], F32, tag="y1")
            nc.vector.tensor_mul(
                y[:, 0:G, :].rearrange("p g (b h) -> p g b h", h=H),
                part4[:, :, 0:32, :],
                _bc(rec[:, 0:G, :].unsqueeze(2), (128, G, 32, H)))
            nc.vector.tensor_add(y[:, 0:G, :], y[:, 0:G, :],
                                 _bc(B1_s[:, :].unsqueeze(1), (128, G, HC1)))
            mn = spool.tile([128, G_MAX, HC1], F32, tag="mn1")
            nc.vector.tensor_scalar_min(mn[:, 0:G, :], y[:, 0:G, :], 0.0)
            nc.vector.tensor_scalar_max(y[:, 0:G, :], y[:, 0:G, :], 0.0)
            em = spool.tile([128, G_MAX, HC1], F32, tag="em1")
            nc.scalar.activation(em[:, 0:G, :], mn[:, 0:G, :], Act.Exp)
            h1t = spool.tile([128, G_MAX, HC1], BF16, tag="h1t")
            nc.vector.scalar_tensor_tensor(h1t[:, 0:G, :], em[:, 0:G, :],
                                           -1.0, y[:, 0:G, :],
                                           op0=Alu.add, op1=Alu.add)
            nc.sync.dma_start(
                h1d[t0 * 128:(t0 + G) * 128, :]
                .rearrange("(g p) c -> p g c", p=128), h1t[:, 0:G, :])

        with tc.tile_pool(name="gp1", bufs=2) as gp1:
            edge_layer(gp1, cfg.SLOT_CAP, pair1, 2 * ROW1, H, 264, 264,
                       ownH1, alL, arL, l1_out)

        # ---- L2 projection (own rank only) + AllGather ----
        with tc.tile_pool(name="ph2", bufs=1) as ph2:
            h1T = ph2.tile([128, KC, NT], BF16, tag="h1T")
            for k in range(KC):
                nc.sync.dma_start_transpose(h1T[:, k, :],
                                            h1d[:, k * 128:(k + 1) * 128])
            tst2 = ph2.tile([128, T, ROW2], BF16, tag="tst2")
            nc.vector.memset(tst2[:, :, 64:128], 0.0)
            nc.vector.memset(tst2[:, :, 64:65], 1.0)
            nc.vector.memset(ownH2[:, :, 64:65], 1.0)
            for t in range(T):
                ps = psum.tile([128, OUT + 2], F32, tag="proj2")
                for k in range(KC):
                    nc.tensor.matmul(ps[:, :],
                                     h1T[:, k, t * 128:(t + 1) * 128],
                                     RHS2[:, k, :], start=(k == 0),
                                     stop=(k == KC - 1))
                nc.scalar.copy(tst2[:, t, 0:OUT], ps[:, 0:OUT])
                nc.scalar.copy(tst2[:, t, 66:67], ps[:, OUT:OUT + 1])
                nc.vector.tensor_copy(al2L[:, t, :], ps[:, OUT:OUT + 1])
                nc.vector.tensor_copy(ar2L[:, t, :],
                                      ps[:, OUT + 1:OUT + 2])
                nc.scalar.copy(ownH2[:, t, 0:64], ps[:, 0:OUT])
            nc.sync.dma_start(
                chunk2[:, :].rearrange("(t n) c -> n t c", n=128),
                tst2[:, :, :])
        nc.gpsimd.collective_compute(
            "AllGather", Alu.bypass,
            replica_groups=[list(range(cfg.R))],
            ins=[chunk2[:, :].opt()], outs=[table2[:, :].opt()])

        # ---- L2 epilogue (batched) ----
        def l2_out(t0, G, part):
            den = part[:, 0:G, 64:65]
            rec = spool.tile([128, G_MAX, 1], F32, tag="rec2")
            nc.vector.reciprocal(rec[:, 0:G, :], den)
            y = spool.tile([128, G_MAX, OUT], F32, tag="y2")
            nc.vector.tensor_mul(y[:, 0:G, :], part[:, 0:G, 0:OUT],
                                 _bc(rec[:, 0:G, :], (128, G, OUT)))
            nc.vector.tensor_add(y[:, 0:G, :], y[:, 0:G, :],
                                 _bc(B2_s[:, :].unsqueeze(1), (128, G, OUT)))
            mx = spool.tile([128, G_MAX, 1], F32, tag="mx2")
            nc.vector.tensor_reduce(mx[:, 0:G, :], y[:, 0:G, :],
                                    axis=mybir.AxisListType.X, op=Alu.max)
            nc.vector.scalar_tensor_tensor(
                y[:, 0:G, :], _bc(mx[:, 0:G, :], (128, G, OUT)), -1.0,
                y[:, 0:G, :], op0=Alu.mult, op1=Alu.add)
            ex = spool.tile([128, G_MAX, OUT], F32, tag="ex2")
            nc.scalar.activation(ex[:, 0:G, :], y[:, 0:G, :], Act.Exp)
            ssum = spool.tile([128, G_MAX, 1], F32, tag="ss2")
            nc.vector.tensor_reduce(ssum[:, 0:G, :], ex[:, 0:G, :],
                                    axis=mybir.AxisListType.X, op=Alu.add)
            ls = spool.tile([128, G_MAX, 1], F32, tag="ls2")
            nc.scalar.activation(ls[:, 0:G, :], ssum[:, 0:G, :], Act.Ln)
            nc.vector.scalar_tensor_tensor(
                ostage[:, t0:t0 + G, :], _bc(ls[:, 0:G, :], (128, G, OUT)),
                -1.0, y[:, 0:G, :], op0=Alu.mult, op1=Alu.add)

        with tc.tile_pool(name="gp2", bufs=2) as gp2:
            edge_layer(gp2, CAP2, pair2, 2 * ROW2, 1, 65, 66, ownH2,
                       al2L, ar2L, l2_out)
        nc.sync.dma_start(out_d.ap().rearrange("(t p) c -> p t c", p=128),
                          ostage[:, :, :])

    nc.compile()
    return nc


def _host_inputs(cfg: Cfg, sch: Sched, inputs: dict):
    x = np.asarray(inputs["x"], np.float32)
    W1 = np.asarray(inputs["W1"], np.float32)
    a1_src = np.asarray(inputs["a1_src"], np.float32)
    a1_dst = np.asarray(inputs["a1_dst"], np.float32)
    b1 = np.asarray(inputs["b1"], np.float32)
    W2 = np.asarray(inputs["W2"], np.float32)
    a2_src = np.asarray(inputs["a2_src"], np.float32)
    a2_dst = np.asarray(inputs["a2_dst"], np.float32)
    b2 = np.asarray(inputs["b2"], np.float32)
    H, HID, HC1, OUT = cfg.HEADS, cfg.HID, cfg.HC1, cfg.OUT

    il = np.empty(HC1, np.int64)               # il[8c+h] = h*32+c
    for c in range(HID):
        for h in range(H):
            il[8 * c + h] = h * HID + c

    Ws = np.zeros((cfg.F_IN, H), np.float32)
    Wd = np.zeros((cfg.F_IN, H), np.float32)
    for h in range(H):
        Ws[:, h] = W1[:, h * HID:(h + 1) * HID] @ a1_src[h]
        Wd[:, h] = W1[:, h * HID:(h + 1) * HID] @ a1_dst[h]
    RHS1 = np.concatenate([W1[:, il], Ws, Wd], axis=1)

    W2p = W2[il, :]
    vs = (W2 @ a2_src.reshape(OUT, 1))[il]
    vd = (W2 @ a2_dst.reshape(OUT, 1))[il]
    RHS2 = np.concatenate([W2p, vs, vd], axis=1)

    xT = np.zeros((cfg.F_IN, cfg.TROWS), np.float32)
    for r in range(cfg.R):
        xT[:, r * cfg.CROWS:r * cfg.CROWS + cfg.NPR] = x[sch.perm[r]].T
    xT16 = xT.astype(ml_dtypes.bfloat16)

    common = {
        "xT": xT16,
        "RHS1": np.ascontiguousarray(RHS1).astype(ml_dtypes.bfloat16),
        "RHS2": np.ascontiguousarray(RHS2).astype(ml_dtypes.bfloat16),
        "B1rep": np.tile(b1[il][None, :], (128, 1)).astype(np.float32),
        "B2rep": np.tile(b2[None, :], (128, 1)).astype(np.float32),
    }
    in_maps = []
    for r in range(cfg.R):
        m = dict(common)
        m["xTo"] = np.ascontiguousarray(
            xT16[:, r * cfg.CROWS:(r + 1) * cfg.CROWS])
        m["idxs"] = np.ascontiguousarray(sch.idx16[r])
        m["mask"] = np.ascontiguousarray(sch.mask[r])
        in_maps.append(m)
    return in_maps


def run(cfg: Cfg, inputs: dict, trace: bool = False):
    edge_index = np.asarray(inputs["edge_index"])
    src = edge_index[0].astype(np.int64)
    dst = edge_index[1].astype(np.int64)

    sch = build_schedule(cfg, src, dst)
    nc = build_program(cfg, sch)
    in_maps = _host_inputs(cfg, sch, inputs)
    res = bass_utils.run_bass_kernel_spmd(
        nc, in_maps, core_ids=list(range(cfg.R)), trace=trace)
    out = np.empty((cfg.N, cfg.OUT), np.float32)
    for r in range(cfg.R):
        o = np.asarray(res.results[r]["out"], np.float32)
        out[sch.perm[r]] = o[:cfg.NPR]
    return out, res


def kernel(**inputs) -> np.ndarray:
    cfg = Cfg()
    out, _ = run(cfg, inputs)
    return out


if __name__ == "__main__":
    import reference
    inputs = {k: np.asarray(v) for k, v in reference.setup_inputs().items()}
    out = kernel(**inputs)
    exp = np.asarray(reference.reference(**reference.setup_inputs()))
    err = np.abs(out - exp).max() / (np.abs(exp).max() + 1e-12)
    print("rel err:", err)


# revision 17
# speedup vs baseline: 1.7093x; 1.0929x over previous
"""2-layer GAT (gnn_message_passing) on 8 TRN2 NeuronCores — v2.

Profiling of v1 showed the kernel is bound by GPSIMD (Q7) descriptor
generation for dma_gather at ~7.4 ns/descriptor; SDMA drain, HBM
bandwidth, PE and DVE all have headroom.  v2 minimizes gather
descriptors and removes the big AllGather:

  - PAIR-ROWS: table rows hold TWO consecutive nodes (1536 B for layer 1,
    512 B for layer 2), so a pair index fits int16 in ONE window
    (25088 < 32768).  The wrong half of each fetched pair is killed by a
    static {0,1} mask multiplied into the attention weight.  This removes
    the lo/hi window split of v1 and its E[max]+E[max] padding
    (870 -> ~507 slots in the tile schedule, -42% descriptors).
  - SELF-LOOPS are never gathered: each rank keeps its own projected rows
    (h + attention logits) resident in SBUF and adds the self
    contribution directly.
  - Layer-1 projection is REPLICATED: x is a replicated input, so every
    rank projects the full table locally; no AllGather for table 1 (the
    layer-2 table is still exchanged with a small ncfw AllGather).
  - Table rows store [h (c-major interleaved) | ones | al(f32)]: the ones
    channels make the softmax denominator fall out of the same tree-sum
    as the numerator, and the attention weight is broadcast-multiplied
    IN-PLACE into the gather buffer (no separate msg buffer, no channel
    broadcast copies).

Both layers share one slot/idx/mask schedule (same edges).
"""

import sys
from contextlib import ExitStack
from dataclasses import dataclass

import numpy as np
import ml_dtypes

for _p in ("/opt/trn_rl_repo",):
    if _p not in sys.path:
        sys.path.insert(0, _p)

import concourse.bass as bass
import concourse.bacc as bacc
import concourse.mybir as mybir
import concourse.tile as tile
from concourse import bass_utils

F32 = mybir.dt.float32
BF16 = mybir.dt.bfloat16
I16 = mybir.dt.int16
Alu = mybir.AluOpType
Act = mybir.ActivationFunctionType


@dataclass
class Cfg:
    N: int = 50000
    E: int = 500000
    F_IN: int = 128
    HID: int = 32
    HEADS: int = 8
    OUT: int = 64
    NEG: float = 0.2
    R: int = 8
    SLOT_CAP: int = 28
    GMAX: int = 4

    @property
    def HC1(self):
        return self.HEADS * self.HID          # 256

    @property
    def NPR(self):
        return self.N // self.R               # 6250

    @property
    def T(self):
        return (self.NPR + 127) // 128        # 49 tiles per rank

    @property
    def CROWS(self):
        return self.T * 128                   # 6272 rows per rank chunk

    @property
    def TROWS(self):
        return self.R * self.CROWS            # 50176

    @property
    def ROW1(self):
        return 384                            # bf16 elems per node sub-row

    @property
    def ROW2(self):
        return 128


# ROW1 sub-row (bf16 elems): [0:256) h interleaved (elem 8c+h), [256:264)
# ones, [264:280) al per head f32, [280:384) pad (junk).
# ROW2 sub-row: [0:64) h2, [64] one, [65] pad, [66:68) al2 f32, pad (junk).


@dataclass
class Sched:
    perm: np.ndarray          # [R, NPR] global node id per (rank, pos)
    D: np.ndarray             # [T] slots per tile
    gso: np.ndarray           # [T] global slot offset of tile
    groups: list              # (t0, t1) with sum(D) <= SLOT_CAP
    sub_off: np.ndarray      # [T] slot offset within its group
    idx16: np.ndarray         # [R, 128, TOTCOL] int16 pair ids
    mask: np.ndarray          # [R, 128, TOTSLOT*2] bf16 {0,1}


def build_schedule(cfg: Cfg, src: np.ndarray, dst: np.ndarray) -> Sched:
    """src/dst: the raw edge list WITHOUT the appended self-loops (the
    kernel's own-row term supplies exactly one self-loop per node; natural
    (i,i) edges in the input stay in the list as regular slots)."""
    N, R, NPR, T = cfg.N, cfg.R, cfg.NPR, cfg.T
    es, ed = src, dst
    deg = np.bincount(ed, minlength=N).astype(np.int64)

    rank_of = np.arange(N) // NPR
    sortpos = np.empty(N, np.int64)
    perm = np.empty((R, NPR), np.int64)
    for r in range(R):
        nodes = np.arange(r * NPR, (r + 1) * NPR)
        order = np.argsort(-deg[nodes], kind="stable")
        perm[r] = nodes[order]
        sortpos[perm[r]] = np.arange(NPR)
    row_of = rank_of * cfg.CROWS + sortpos     # [N] global table row

    degs_sorted = deg[perm]                    # [R, NPR]
    Dt = np.zeros(T, np.int64)
    for t in range(T):
        sl = slice(t * 128, min((t + 1) * 128, NPR))
        Dt[t] = max(1, degs_sorted[:, sl].max())
    assert Dt.max() <= cfg.SLOT_CAP, f"{Dt.max()} > {cfg.SLOT_CAP}"
    assert cfg.TROWS // 2 < 32768

    # equal-D batches: G consecutive tiles padded to the first tile's D so
    # one vector instruction can span all G tiles with regular APs
    groups = []
    D = np.zeros(T, np.int64)
    t0 = 0
    while t0 < T:
        Dm, G = Dt[t0], 1
        while (t0 + G < T and G < cfg.GMAX
               and (G + 1) * Dm <= cfg.SLOT_CAP):
            G += 1
        D[t0:t0 + G] = Dm
        groups.append((t0, t0 + G))
        t0 += G
    gso = np.concatenate([[0], np.cumsum(D)])[:-1]
    TOTSLOT = int(D.sum())
    sub_off = np.array([gso[t] - gso[g0] for (g0, g1) in groups
                        for t in range(g0, g1)], np.int64)

    dkey = rank_of[ed] * NPR + sortpos[ed]
    order = np.argsort(dkey, kind="stable")
    s_row = row_of[es[order]]
    s_dkey = dkey[order]
    cnt = np.bincount(dkey, minlength=R * NPR)
    start = np.concatenate([[0], np.cumsum(cnt)])[:-1]
    slot = np.arange(len(order)) - start[s_dkey]

    e_rank = s_dkey // NPR
    e_pos = s_dkey % NPR
    e_tile = e_pos // 128
    e_part = e_pos % 128

    TOTCOL = TOTSLOT * 8
    idx16 = np.empty((R, 128, TOTCOL), np.int16)
    mask = np.zeros((R, 128, TOTSLOT * 2), np.float32)
    for r in range(R):
        vals = np.zeros(TOTSLOT * 128, np.int64)
        m = e_rank == r
        pos = gso[e_tile[m]] * 128 + slot[m] * 128 + e_part[m]
        vals[pos] = s_row[m] >> 1
        assert vals.max() < 32768
        idx16[r] = np.tile(vals.astype(np.int16).reshape(-1, 16).T, (8, 1))
        half = (s_row[m] & 1).astype(np.int64)
        mcol = (gso[e_tile[m]] + slot[m]) * 2 + half
        mask[r, e_part[m], mcol] = 1.0
    return Sched(perm=perm, D=D, gso=gso, groups=groups, sub_off=sub_off,
                 idx16=idx16,
                 mask=mask.astype(ml_dtypes.bfloat16))


def _bc(ap, shape):
    return ap.broadcast_to(list(shape))


def build_program(cfg: Cfg, sch: Sched):
    nc = bacc.Bacc("TRN2", target_bir_lowering=False, debug=False,
                   num_devices=cfg.R)
    T, R, TROWS, CROWS = cfg.T, cfg.R, cfg.TROWS, cfg.CROWS
    HC1, H, OUT = cfg.HC1, cfg.HEADS, cfg.OUT
    ROW1, ROW2 = cfg.ROW1, cfg.ROW2
    TOTCOL = sch.idx16.shape[2]
    TOTSLOT = sch.mask.shape[2] // 2
    NT = T * 128
    KC = HC1 // 128
    G_MAX = cfg.GMAX
    CAP2 = cfg.SLOT_CAP

    xT_d = nc.dram_tensor("xT", [cfg.F_IN, TROWS], BF16, kind="ExternalInput")
    xTo_d = nc.dram_tensor("xTo", [cfg.F_IN, CROWS], BF16,
                           kind="ExternalInput")
    idx_d = nc.dram_tensor("idxs", [128, TOTCOL], I16, kind="ExternalInput")
    msk_d = nc.dram_tensor("mask", [128, TOTSLOT * 2], BF16,
                           kind="ExternalInput")
    RHS1_d = nc.dram_tensor("RHS1", [128, HC1 + 2 * H], BF16,
                            kind="ExternalInput")
    RHS2_d = nc.dram_tensor("RHS2", [HC1, OUT + 2], BF16,
                            kind="ExternalInput")
    B1_d = nc.dram_tensor("B1rep", [128, HC1], F32, kind="ExternalInput")
    B2_d = nc.dram_tensor("B2rep", [128, OUT], F32, kind="ExternalInput")
    out_d = nc.dram_tensor("out", [NT, OUT], F32, kind="ExternalOutput")

    with tile.TileContext(nc) as tc, ExitStack() as ctx:
        dram = ctx.enter_context(tc.tile_pool(name="dram", bufs=1,
                                              space="DRAM"))
        const = ctx.enter_context(tc.tile_pool(name="const", bufs=1))
        psum = ctx.enter_context(tc.tile_pool(name="psum", bufs=2,
                                              space="PSUM"))

        table1 = dram.tile([TROWS, ROW1], BF16)
        h1d = dram.tile([NT, HC1], BF16)
        chunk2 = dram.tile([CROWS, ROW2], BF16)
        table2 = dram.tile([TROWS, ROW2], BF16, addr_space="Shared")

        idx_s = const.tile([128, TOTCOL], I16, tag="idx")
        nc.sync.dma_start(idx_s[:, :], idx_d[:, :])
        msk_s = const.tile([128, TOTSLOT * 2], BF16, tag="msk")
        nc.sync.dma_start(msk_s[:, :], msk_d[:, :])
        B1_s = const.tile([128, HC1], F32, tag="b1")
        nc.sync.dma_start(B1_s[:, :], B1_d[:, :])
        B2_s = const.tile([128, OUT], F32, tag="b2")
        nc.sync.dma_start(B2_s[:, :], B2_d[:, :])
        RHS1 = const.tile([128, HC1 + 2 * H], BF16, tag="rhs1")
        nc.sync.dma_start(RHS1[:, :], RHS1_d[:, :])
        RHS2 = const.tile([128, KC, OUT + 2], BF16, tag="rhs2")
        nc.sync.dma_start(RHS2[:, :, :],
                          RHS2_d.ap().rearrange("(k p) c -> p k c", p=128))

        # per-rank own-node state for self-loop contributions
        ownH1 = const.tile([128, T, 264], BF16, tag="ownh1")
        alL = const.tile([128, T, H], F32, tag="alL")
        arL = const.tile([128, T, H], F32, tag="arL")
        ownH2 = const.tile([128, T, 65], BF16, tag="ownh2")
        al2L = const.tile([128, T, 1], F32, tag="al2L")
        ar2L = const.tile([128, T, 1], F32, tag="ar2L")
        proj2S = const.tile([128, T, OUT + 2], F32, tag="proj2s")

        # ============ phase 1: projections -> table 1 (replicated) ========
        with tc.tile_pool(name="ph1", bufs=2) as ph1:
            # own-chunk pass: fill ownH1 / alL / arL
            xo = ph1.tile([128, T, 128], BF16, tag="xo")
            nc.sync.dma_start(xo[:, :, :],
                              xTo_d.ap().rearrange("p (t n) -> p t n", n=128))
            nc.vector.memset(ownH1[:, :, 256:264], 1.0)
            for t in range(T):
                ps = psum.tile([128, HC1 + 2 * H], F32, tag="proj0")
                nc.tensor.matmul(ps[:, :], xo[:, t, :], RHS1[:, :],
                                 start=True, stop=True)
                nc.scalar.copy(ownH1[:, t, 0:HC1], ps[:, 0:HC1])
                nc.vector.tensor_copy(alL[:, t, :], ps[:, HC1:HC1 + H])
                nc.vector.tensor_copy(arL[:, t, :],
                                      ps[:, HC1 + H:HC1 + 2 * H])
            # full-table pass (all ranks compute everything)
            for q in range(R):
                xq = ph1.tile([128, T, 128], BF16, tag="xq")
                nc.sync.dma_start(
                    xq[:, :, :],
                    xT_d[:, q * CROWS:(q + 1) * CROWS]
                    .rearrange("p (t n) -> p t n", n=128))
                tst = ph1.tile([128, T, ROW1], BF16, tag="tst")
                nc.vector.memset(tst[:, :, 256:264], 1.0)
                for t in range(T):
                    ps = psum.tile([128, HC1 + 2 * H], F32, tag="proj1")
                    nc.tensor.matmul(ps[:, :], xq[:, t, :], RHS1[:, :],
                                     start=True, stop=True)
                    nc.scalar.copy(tst[:, t, 0:HC1], ps[:, 0:HC1])
                    al_v = tst[:, t, 264:280].bitcast(F32)
                    nc.vector.tensor_copy(al_v[:, :], ps[:, HC1:HC1 + H])
                nc.sync.dma_start(
                    table1[q * CROWS:(q + 1) * CROWS, :]
                    .rearrange("(t n) c -> n t c", n=128),
                    tst[:, :, :])

        pair1 = table1[:, :].rearrange("(a b) c -> a (b c)", b=2)
        pair2 = table2[:, :].rearrange("(a b) c -> a (b c)", b=2)

        spool = ctx.enter_context(tc.tile_pool(name="spool", bufs=2))

        # ================= edge phase (both layers) =================
        def edge_layer(gpool, CAP, pair_tab, ROW, NH, CHB, al_off, own,
                       alo, aro, out_cb):
            """Batched over equal-D tile groups.  ROW: pair elems; CHB:
            payload elems per sub-row (h+ones); al_off: bf16-elem offset of
            al in a sub-row; own: [128, T, CHB] bf16; alo/aro: [128, T, NH]
            bf16."""
            SUB = ROW // 2
            for (t0, t1) in sch.groups:
                G = t1 - t0
                Dm = int(sch.D[t0])
                S = G * Dm                    # gathered pairs
                S2, D2 = 2 * S, 2 * Dm
                b0 = int(sch.gso[t0])
                g = gpool.tile([128, CAP, ROW], BF16, tag="g")
                nc.gpsimd.dma_gather(
                    g[:, 0:S, :], pair_tab, idx_s[:, 8 * b0:8 * (b0 + S)],
                    num_idxs=S * 128, num_idxs_reg=S * 128,
                    elem_size=ROW, elem_step=ROW, single_packet=False)
                gs = g[:, 0:S, :]
                sub = gs.rearrange("p s (x c) -> p (s x) c", x=2)
                al = gs.bitcast(F32).rearrange(
                    "p s (x c) -> p (s x) c", x=2)[:, :, al_off:al_off + NH]
                e = spool.tile([128, 2 * CAP, NH], F32, tag="e")
                nc.vector.tensor_add(
                    e[:, 0:S2, :].rearrange("p (g d) h -> p g d h", g=G),
                    al.rearrange("p (g d) h -> p g d h", g=G),
                    _bc(aro[:, t0:t1, :].unsqueeze(2), (128, G, D2, NH)))
                nc.vector.scalar_tensor_tensor(
                    e[:, 0:S2, :], e[:, 0:S2, :], cfg.NEG, e[:, 0:S2, :],
                    op0=Alu.mult, op1=Alu.max)
                p = spool.tile([128, 2 * CAP, NH], BF16, tag="p")
                nc.scalar.activation(p[:, 0:S2, :], e[:, 0:S2, :], Act.Exp)
                mk = msk_s[:, 2 * b0:2 * (b0 + S)]
                nc.vector.tensor_mul(
                    p[:, 0:S2, :], p[:, 0:S2, :],
                    _bc(mk.unsqueeze(2), (128, S2, NH)))
                msg4 = sub[:, :, 0:CHB].rearrange("p s (b h) -> p s b h",
                                                  h=NH)
                nc.vector.tensor_mul(
                    msg4, msg4,
                    _bc(p[:, 0:S2, :].unsqueeze(2),
                        (128, S2, CHB // NH, NH)))
                # self contribution
                es_ = spool.tile([128, G_MAX, NH], F32, tag="es")
                nc.vector.tensor_add(es_[:, 0:G, :], alo[:, t0:t1, :],
                                     aro[:, t0:t1, :])
                nc.vector.scalar_tensor_tensor(
                    es_[:, 0:G, :], es_[:, 0:G, :], cfg.NEG, es_[:, 0:G, :],
                    op0=Alu.mult, op1=Alu.max)
                ps_ = spool.tile([128, G_MAX, NH], F32, tag="psx")
                nc.scalar.activation(ps_[:, 0:G, :], es_[:, 0:G, :], Act.Exp)
                selfh = spool.tile([128, G_MAX, CHB], F32, tag="selfh")
                nc.scalar.copy(selfh[:, 0:G, :], own[:, t0:t1, :])
                part = spool.tile([128, G_MAX, CHB], F32, tag="part")
                nc.vector.tensor_mul(
                    part[:, 0:G, :].rearrange("p g (b h) -> p g b h", h=NH),
                    selfh[:, 0:G, :].rearrange("p g (b h) -> p g b h", h=NH),
                    _bc(ps_[:, 0:G, :].unsqueeze(2),
                        (128, G, CHB // NH, NH)))
                # tree-sum over D2 slots within each tile, all tiles at once
                mv = sub[:, :, 0:CHB].rearrange("p (g d) c -> p g d c", g=G)
                cur = D2
                while cur > 1:
                    hh = cur // 2
                    nc.vector.tensor_add(mv[:, :, 0:hh, :], mv[:, :, 0:hh, :],
                                         mv[:, :, cur - hh:cur, :])
                    cur -= hh
                nc.scalar.copy(selfh[:, 0:G, :], mv[:, :, 0, :])
                nc.vector.tensor_add(part[:, 0:G, :], part[:, 0:G, :],
                                     selfh[:, 0:G, :])
                out_cb(t0, G, part)

        # ---- L1 epilogue (batched) ----
        def l1_out(t0, G, part):
            part4 = part[:, 0:G, :].rearrange("p g (b h) -> p g b h", h=H)
            den = part4[:, :, 32, :]
            rec = spool.tile([128, G_MAX, H], F32, tag="rec")
            nc.vector.reciprocal(rec[:, 0:G, :], den)
            y = spool.tile([128, G_MAX, HC1], F32, tag="y1")
            nc.vector.tensor_mul(
                y[:, 0:G, :].rearrange("p g (b h) -> p g b h", h=H),
                part4[:, :, 0:32, :],
                _bc(rec[:, 0:G, :].unsqueeze(2), (128, G, 32, H)))
            nc.vector.tensor_add(y[:, 0:G, :], y[:, 0:G, :],
                                 _bc(B1_s[:, :].unsqueeze(1), (128, G, HC1)))
            mn = spool.tile([128, G_MAX, HC1], F32, tag="mn1")
            nc.vector.tensor_scalar_min(mn[:, 0:G, :], y[:, 0:G, :], 0.0)
            nc.vector.tensor_scalar_max(y[:, 0:G, :], y[:, 0:G, :], 0.0)
            em = spool.tile([128, G_MAX, HC1], F32, tag="em1")
            nc.scalar.activation(em[:, 0:G, :], mn[:, 0:G, :], Act.Exp)
            h1t = spool.tile([128, G_MAX, HC1], BF16, tag="h1t")
            nc.vector.scalar_tensor_tensor(h1t[:, 0:G, :], em[:, 0:G, :],
                                           -1.0, y[:, 0:G, :],
                                           op0=Alu.add, op1=Alu.add)
            nc.sync.dma_start(
                h1d[t0 * 128:(t0 + G) * 128, :]
                .rearrange("(g p) c -> p g c", p=128), h1t[:, 0:G, :])
            # layer-2 projection for these tiles (PE/scalar, off the
            # contended engines; overlaps the remaining layer-1 batches)
            h1Tb = spool.tile([128, KC, G_MAX * 128], BF16, tag="h1Tb")
            for k in range(KC):
                nc.sync.dma_start_transpose(
                    h1Tb[:, k, 0:G * 128],
                    h1d[t0 * 128:(t0 + G) * 128, k * 128:(k + 1) * 128])
            for j in range(G):
                ps2 = psum.tile([128, OUT + 2], F32, tag="proj2")
                for k in range(KC):
                    nc.tensor.matmul(ps2[:, :],
                                     h1Tb[:, k, j * 128:(j + 1) * 128],
                                     RHS2[:, k, :], start=(k == 0),
                                     stop=(k == KC - 1))
                nc.scalar.copy(proj2S[:, t0 + j, :], ps2[:, :])

        with tc.tile_pool(name="gp1", bufs=2) as gp1:
            edge_layer(gp1, cfg.SLOT_CAP, pair1, 2 * ROW1, H, 264, 132,
                       ownH1, alL, arL, l1_out)

        # ---- stitch layer-2 table chunk + AllGather ----
        with tc.tile_pool(name="ph2", bufs=1) as ph2:
            tst2 = ph2.tile([128, T, ROW2], BF16, tag="tst2")
            nc.vector.memset(tst2[:, :, 64:128], 0.0)
            nc.vector.memset(tst2[:, :, 64:65], 1.0)
            nc.vector.memset(ownH2[:, :, 64:65], 1.0)
            nc.scalar.copy(tst2[:, :, 0:OUT], proj2S[:, :, 0:OUT])
            al2_v = tst2[:, :, 66:68].bitcast(F32)
            nc.vector.tensor_copy(al2_v[:, :, :], proj2S[:, :, 64:65])
            nc.vector.tensor_copy(al2L[:, :, :], proj2S[:, :, 64:65])
            nc.vector.tensor_copy(ar2L[:, :, :], proj2S[:, :, 65:66])
            nc.scalar.copy(ownH2[:, :, 0:64], proj2S[:, :, 0:OUT])
            nc.sync.dma_start(
                chunk2[:, :].rearrange("(t n) c -> n t c", n=128),
                tst2[:, :, :])
        nc.gpsimd.collective_compute(
            "AllGather", Alu.bypass,
            replica_groups=[list(range(cfg.R))],
            ins=[chunk2[:, :].opt()], outs=[table2[:, :].opt()])

        # ---- L2 epilogue (batched) ----
        def l2_out(t0, G, part):
            den = part[:, 0:G, 64:65]
            rec = spool.tile([128, G_MAX, 1], F32, tag="rec2")
            nc.vector.reciprocal(rec[:, 0:G, :], den)
            y = spool.tile([128, G_MAX, OUT], F32, tag="y2")
            nc.vector.tensor_mul(y[:, 0:G, :], part[:, 0:G, 0:OUT],
                                 _bc(rec[:, 0:G, :], (128, G, OUT)))
            nc.vector.tensor_add(y[:, 0:G, :], y[:, 0:G, :],
                                 _bc(B2_s[:, :].unsqueeze(1), (128, G, OUT)))
            mx = spool.tile([128, G_MAX, 1], F32, tag="mx2")
            nc.vector.tensor_reduce(mx[:, 0:G, :], y[:, 0:G, :],
                                    axis=mybir.AxisListType.X, op=Alu.max)
            nc.vector.scalar_tensor_tensor(
                y[:, 0:G, :], _bc(mx[:, 0:G, :], (128, G, OUT)), -1.0,
                y[:, 0:G, :], op0=Alu.mult, op1=Alu.add)
            ex = spool.tile([128, G_MAX, OUT], F32, tag="ex2")
            nc.scalar.activation(ex[:, 0:G, :], y[:, 0:G, :], Act.Exp)
            ssum = spool.tile([128, G_MAX, 1], F32, tag="ss2")
            nc.vector.tensor_reduce(ssum[:, 0:G, :], ex[:, 0:G, :],
                                    axis=mybir.AxisListType.X, op=Alu.add)
            ls = spool.tile([128, G_MAX, 1], F32, tag="ls2")
            nc.scalar.activation(ls[:, 0:G, :], ssum[:, 0:G, :], Act.Ln)
            o = spool.tile([128, G_MAX, OUT], F32, tag="o2")
            nc.vector.scalar_tensor_tensor(
                o[:, 0:G, :], _bc(ls[:, 0:G, :], (128, G, OUT)),
                -1.0, y[:, 0:G, :], op0=Alu.mult, op1=Alu.add)
            nc.sync.dma_start(
                out_d[t0 * 128:(t0 + G) * 128, :]
                .rearrange("(g p) c -> p g c", p=128), o[:, 0:G, :])

        with tc.tile_pool(name="gp2", bufs=3) as gp2:
            edge_layer(gp2, CAP2, pair2, 2 * ROW2, 1, 65, 33, ownH2,
                       al2L, ar2L, l2_out)

    nc.compile()
    return nc


def _host_inputs(cfg: Cfg, sch: Sched, inputs: dict):
    x = np.asarray(inputs["x"], np.float32)
    W1 = np.asarray(inputs["W1"], np.float32)
    a1_src = np.asarray(inputs["a1_src"], np.float32)
    a1_dst = np.asarray(inputs["a1_dst"], np.float32)
    b1 = np.asarray(inputs["b1"], np.float32)
    W2 = np.asarray(inputs["W2"], np.float32)
    a2_src = np.asarray(inputs["a2_src"], np.float32)
    a2_dst = np.asarray(inputs["a2_dst"], np.float32)
    b2 = np.asarray(inputs["b2"], np.float32)
    H, HID, HC1, OUT = cfg.HEADS, cfg.HID, cfg.HC1, cfg.OUT

    il = np.empty(HC1, np.int64)               # il[8c+h] = h*32+c
    for c in range(HID):
        for h in range(H):
            il[8 * c + h] = h * HID + c

    Ws = np.zeros((cfg.F_IN, H), np.float32)
    Wd = np.zeros((cfg.F_IN, H), np.float32)
    for h in range(H):
        Ws[:, h] = W1[:, h * HID:(h + 1) * HID] @ a1_src[h]
        Wd[:, h] = W1[:, h * HID:(h + 1) * HID] @ a1_dst[h]
    RHS1 = np.concatenate([W1[:, il], Ws, Wd], axis=1)

    W2p = W2[il, :]
    vs = (W2 @ a2_src.reshape(OUT, 1))[il]
    vd = (W2 @ a2_dst.reshape(OUT, 1))[il]
    RHS2 = np.concatenate([W2p, vs, vd], axis=1)

    xT = np.zeros((cfg.F_IN, cfg.TROWS), np.float32)
    for r in range(cfg.R):
        xT[:, r * cfg.CROWS:r * cfg.CROWS + cfg.NPR] = x[sch.perm[r]].T
    xT16 = xT.astype(ml_dtypes.bfloat16)

    common = {
        "xT": xT16,
        "RHS1": np.ascontiguousarray(RHS1).astype(ml_dtypes.bfloat16),
        "RHS2": np.ascontiguousarray(RHS2).astype(ml_dtypes.bfloat16),
        "B1rep": np.tile(b1[il][None, :], (128, 1)).astype(np.float32),
        "B2rep": np.tile(b2[None, :], (128, 1)).astype(np.float32),
    }
    in_maps = []
    for r in range(cfg.R):
        m = dict(common)
        m["xTo"] = np.ascontiguousarray(
            xT16[:, r * cfg.CROWS:(r + 1) * cfg.CROWS])
        m["idxs"] = np.ascontiguousarray(sch.idx16[r])
        m["mask"] = np.ascontiguousarray(sch.mask[r])
        in_maps.append(m)
    return in_maps


def run(cfg: Cfg, inputs: dict, trace: bool = False):
    edge_index = np.asarray(inputs["edge_index"])
    src = edge_index[0].astype(np.int64)
    dst = edge_index[1].astype(np.int64)

    sch = build_schedule(cfg, src, dst)
    nc = build_program(cfg, sch)
    in_maps = _host_inputs(cfg, sch, inputs)
    res = bass_utils.run_bass_kernel_spmd(
        nc, in_maps, core_ids=list(range(cfg.R)), trace=trace)
    out = np.empty((cfg.N, cfg.OUT), np.float32)
    for r in range(cfg.R):
        o = np.asarray(res.results[r]["out"], np.float32)
        out[sch.perm[r]] = o[:cfg.NPR]
    return out, res


def kernel(**inputs) -> np.ndarray:
    cfg = Cfg()
    out, _ = run(cfg, inputs)
    return out


if __name__ == "__main__":
    import reference
    inputs = {k: np.asarray(v) for k, v in reference.setup_inputs().items()}
    out = kernel(**inputs)
    exp = np.asarray(reference.reference(**reference.setup_inputs()))
    err = np.abs(out - exp).max() / (np.abs(exp).max() + 1e-12)
    print("rel err:", err)
